# revision 71
# baseline (speedup 1.0000x reference)
"""Trainium2 Bass kernel: sliding-window multihead attention w/ ALiBi.

Computation (per reference):
  qkv = x @ w_in.T ; q,k,v heads ; blocked sliding-window causal attention
  (window=512, ALiBi bias slope_h*(q_idx-kv_idx)) ; out = o @ w_out.T

Sharding: 8 cores = 4 batches x 2 head-groups (8 heads each). Each core
computes its batch's QKV for its heads, attention, and a partial out-proj
over its heads' columns. Host sums the two head-group partials per batch.

Softmax trick: P = exp(s_raw) * EXPBIG where EXPBIG = exp(bias - bound)
is a host-precomputed Toeplitz band (exact 0 outside the valid window).
The row-max subtraction is replaced by a static bound folded into EXPBIG
(block 0 uses a per-partition ACT bias instead). Each head's PV stationary
is [v (64 cols) | ones (64 cols)], so the PV matmul lands the softmax
denominator replicated across PSUM rows 64:128 and a single DVE divide
per half produces the normalized output tile.
"""

import os
import numpy as np
import ml_dtypes
from contextlib import ExitStack

import concourse.bass as bass
import concourse.bacc as bacc
import concourse.tile as tile
import concourse.mybir as mybir
from concourse.bass_utils import run_bass_kernel_spmd

F16 = mybir.dt.float16
BF16 = mybir.dt.bfloat16
F32 = mybir.dt.float32
AF = mybir.ActivationFunctionType
ALU = mybir.AluOpType

B, S, E = 4, 2048, 1024
H, D, WIN = 16, 64, 512
NB = S // WIN          # 4 blocks
HPC = 8                # heads per core
NCORES = 8
CM = 6.0               # softmax bound safety margin

LAST_RESULTS = None


def _qrange(jt):
    # valid q-column range for scores j-tile jt (window band)
    lo = max(0, 128 * jt - 512)
    hi = min(512, 128 * jt + 128)
    return lo, hi - lo


def _build_nc():
    nc = bacc.Bacc("TRN2", target_bir_lowering=False, debug=False,
                   num_devices=NCORES)

    # host pre-packs every tensor in its SBUF layout ([128, k, cols]) so
    # each one loads with a single large DMA: per-queue dma_start issue
    # overhead (~0.9us each) was the real input-phase bottleneck
    # xT packed [p, colblock, k, 512]: per-partition lines are 8KB contiguous
    # (k-major inside a column block), so each col-block loads as one fast DMA
    xT = nc.dram_tensor("xT", [128, 4, 8, 512], F16,
                        kind="ExternalInput").ap()
    wqk = nc.dram_tensor("w_qk", [128, 8, 1024], F16,
                         kind="ExternalInput").ap()
    wv = nc.dram_tensor("w_v", [128, 8, 512], F16, kind="ExternalInput").ap()
    wo = nc.dram_tensor("w_o", [128, 4, 1024], F16,
                        kind="ExternalInput").ap()
    ebig = nc.dram_tensor("expbig", [128, 4, 2816], BF16,
                          kind="ExternalInput").ap()
    em0 = nc.dram_tensor("em0", [128, 1024], BF16, kind="ExternalInput").ap()
    b0v = nc.dram_tensor("b0v", [128, 32], F32, kind="ExternalInput").ap()
    outp = nc.dram_tensor("out_p", [S, E], F16, kind="ExternalOutput").ap()

    with tile.TileContext(nc) as tc, ExitStack() as ctx:
        pp = ctx.enter_context(tc.tile_pool(name="persist", bufs=1))

        # persistent SBUF tensors
        qkT = [pp.tile([128, S], F16, name=f"qkT{m}", tag=f"qkT{m}")
               for m in range(8)]                       # f-major qk.T
        VA = [pp.tile([128, HPC * 128], BF16, name=f"VA{s}", tag=f"VA{s}")
              for s in range(16)]                  # per head: v(64) | ones(64)
        OT = [pp.tile([128, 512], F16, name=f"OT{i}", tag=f"OT{i}")
              for i in range(16)]                       # normalized o.T
        EBB = pp.tile([128, 4 * 2816], BF16, name="EBB", tag="EBB")
        EBv4 = EBB.rearrange("p (h c) -> p h c", h=4)
        EB = [EBv4[:, h, :] for h in range(4)]     # exp(bias-bound) band pairs
        EM = pp.tile([128, 1024], BF16, name="EM", tag="EM")  # blk0 causal 0/1
        B0 = pp.tile([128, 32], F32, name="B0", tag="B0")    # blk0 exp biases
        WOB = pp.tile([128, 4 * 1024], F16, name="WOB", tag="WOB")
        WOv = WOB.rearrange("p (k c) -> p k c", k=4)
        WO = [WOv[:, k, :] for k in range(4)]

        with tc.tile_pool(name="phA", bufs=1) as pa, \
             tc.tile_pool(name="Pp", bufs=8) as Ppool, \
             tc.tile_pool(name="aps", bufs=2, space="PSUM") as aps:
            XTB = pa.tile([128, 8 * S], F16, name="XTB", tag="XTB")
            XTv = XTB.rearrange("p (cb k r) -> p cb k r", cb=4, k=8)

            def xts(kt, c0, w):
                # xT column range [c0, c0+w) of chunk kt; must stay inside
                # one 512-col block
                cb, r = divmod(c0, 512)
                return XTv[:, cb, kt, r:r + w]
            WQKB = pa.tile([128, 8 * 1024], F16, name="WQKB", tag="WQKB")
            WQKv = WQKB.rearrange("p (k c) -> p k c", k=8)
            wqks = [WQKv[:, k, :] for k in range(8)]
            WVB = pa.tile([128, 8 * 512], F16, name="WVB", tag="WVB")
            WVv = WVB.rearrange("p (k c) -> p k c", k=8)
            wvs = [WVv[:, k, :] for k in range(8)]
            wrm = pa.tile([128, 256], F16, name="wrm", tag="wrm")

            # warm-up weights first so the PE can start ramping immediately
            nc.gpsimd.memset(wrm[:], 0.0)

            # -- input DMA: one large transfer per tensor / 512-col xT slice
            # (per-dma_start issue overhead ~0.9us was the input bottleneck;
            # host pre-packs so per-partition lines are >=8KB contiguous).
            # Single prioritized FIFO on the sync queue in strict need-order:
            # one queue alone sustains ~400GB/s, and parallel queues would
            # fair-share the bus and starve the prefix-critical transfers.
            nc.sync.dma_start(WVv[:], wv[:])
            nc.sync.dma_start(XTv[:, 0], xT[:, 0])
            nc.sync.dma_start(WQKv[:], wqk[:])
            nc.sync.dma_start(XTv[:, 1], xT[:, 1])
            nc.sync.dma_start(XTv[:, 2], xT[:, 2])
            nc.sync.dma_start(XTv[:, 3], xT[:, 3])
            nc.sync.dma_start(WOv[:], wo[:])
            nc.scalar.dma_start(B0[:], b0v[:])
            # EM now; EB bands are issued lazily inside the blk0 iterations
            # (one per head-pair) so they never starve the prefix transfers
            nc.gpsimd.dma_start(EM[:], em0[:])

            # HAM warm-up: dummy matmuls ramp the PE p-state while DMAs land
            wps = aps.tile([128, 512], F32, name="wps", tag="projch", bufs=2)
            NWARM = 16
            for i in range(NWARM):
                nc.tensor.matmul(wps[:, 0:128], wrm[:, 0:128],
                                 wrm[:, 0:128],
                                 start=(i == 0), stop=(i == NWARM - 1))
            # DMA-paced warm-up: dummy matmuls gated on arriving wv chunks
            # keep a low-duty PE heartbeat through the DMA-fill window, so
            # the activity manager grants full clock before the projection
            # phase without the burst-then-claw-back pattern
            wps2 = aps.tile([128, 512], F32, name="wps2", tag="projch",
                            bufs=2)
            for k in range(8):
                nc.tensor.matmul(wps2[:], wvs[k][:, 0:128],
                                 wvs[k][:],
                                 start=(k == 0), stop=(k == 7))

            # ones columns for the denominator replicas (v halves get
            # overwritten by proj_b)
            for st in range(16):
                ones_v = VA[st].rearrange("p (h c) -> p h c", h=HPC)
                nc.gpsimd.memset(ones_v[:, :, 64:128], 1.0)

            # ---- projection b chunk: v[s, f] into VA v-halves -------------
            def proj_b_chunk(st, dve_copy=False):
                pv = aps.tile([128, 512], F32, name=f"pv{st}", tag="projch",
                              bufs=2)
                for kt in range(8):
                    nc.tensor.matmul(
                        pv[:],
                        xts(kt, 128 * st, 128),
                        wvs[kt][:],
                        start=(kt == 0), stop=(kt == 7))
                src = pv.rearrange("p (h c) -> p h c", h=HPC)
                dst = VA[st].rearrange("p (h c) -> p h c", h=HPC)
                if dve_copy:
                    nc.vector.tensor_copy(dst[:, :, 0:64], src[:])
                else:
                    nc.scalar.activation(dst[:, :, 0:64], src[:], AF.Copy)

            # ---- projection a: qkT[f, s], one (mt, sc) chunk at a time ----
            def proj_a_chunk(mt, sc):
                ps = aps.tile([128, 512], F32, name=f"pa{mt}_{sc}",
                              tag="projch", bufs=2)
                for kt in range(8):
                    nc.tensor.matmul(
                        ps[:],
                        wqks[kt][:, 128 * mt:128 * (mt + 1)],
                        xts(kt, 512 * sc, 512),
                        start=(kt == 0), stop=(kt == 7))
                # split the PSUM->SBUF casts across ACT and DVE to balance
                # engine load (GPSIMD cannot read PSUM)
                if (mt + sc) % 2 == 0:
                    nc.scalar.activation(qkT[mt][:, 512 * sc:512 * (sc + 1)],
                                         ps[:], AF.Copy)
                else:
                    nc.vector.tensor_copy(qkT[mt][:, 512 * sc:512 * (sc + 1)],
                                          ps[:])

            # ---- split-K out-projection: open partial sums over a subset of
            # head-pairs (their OT blocks are ready early), close with the
            # rest once the final head-pair lands ----
            po_hold = {}

            def outproj_open(st, kts, tag):
                blk_, qq = st // 4, st % 4
                if tag == "S":
                    po = aps.tile([128, 1024], F32, name=f"poH{st}", tag="S")
                    halves = [po[:, 0:512], po[:, 512:1024]]
                else:
                    halves = [aps.tile([128, 512], F32, name=f"poH{st}_{h}",
                                       tag="projch", bufs=2)[:]
                              for h in range(2)]
                po_hold[st] = halves
                for half in range(2):
                    for i, kt in enumerate(kts):
                        nc.tensor.matmul(
                            halves[half],
                            OT[4 * kt + blk_][:, 128 * qq:128 * (qq + 1)],
                            WO[kt][:, 512 * half:512 * half + 512],
                            start=(i == 0), stop=False,
                            skip_group_check=True)

            def outproj_close(st, kts):
                blk_, qq = st // 4, st % 4
                halves = po_hold[st]
                stg = pa.tile([128, 1024], F16, name=f"stgH{st}",
                              tag="stg", bufs=3)
                for half in range(2):
                    for i, kt in enumerate(kts):
                        nc.tensor.matmul(
                            halves[half],
                            OT[4 * kt + blk_][:, 128 * qq:128 * (qq + 1)],
                            WO[kt][:, 512 * half:512 * half + 512],
                            start=False, stop=(i == len(kts) - 1),
                            skip_group_check=True)
                    # tail: split the PSUM->SBUF casts across ACT and DVE
                    # so they drain in parallel after the last matmul
                    dst = stg[:, 512 * half:512 * half + 512]
                    if half == 0:
                        nc.scalar.activation(dst, halves[half], AF.Copy)
                    else:
                        nc.vector.tensor_copy(dst, halves[half])
                # one full-row 2KB-line DMA; strictly alternate queues in
                # emission order (14,12,13 -> sync,scalar,sync) so the tail
                # drains evenly on both
                eng = nc.scalar if st == 12 else nc.sync
                eng.dma_start(outp[128 * st:128 * (st + 1), :], stg[:])

            # ---- out-projection chunk (one s-tile) ----
            def outproj_chunk(st, split_stg=False):
                blk_, qq = st // 4, st % 4
                # both halves stage into one [128, 2KB] tile -> a single
                # full-row DMA (2KB lines = half the packets of 1KB lines)
                stg = pa.tile([128, 1024], F16, name=f"stg{st}",
                              tag="stg", bufs=3)
                for half in range(2):
                    po = aps.tile([128, 512], F32, name=f"po{st}_{half}",
                                  tag="projch", bufs=2)
                    for kt in range(4):
                        nc.tensor.matmul(
                            po[:],
                            OT[4 * kt + blk_][:, 128 * qq:128 * (qq + 1)],
                            WO[kt][:, 512 * half:512 * (half + 1)],
                            start=(kt == 0), stop=(kt == 3))
                    dst = stg[:, 512 * half:512 * (half + 1)]
                    if split_stg and half == 1:
                        nc.vector.tensor_copy(dst, po[:])
                    else:
                        nc.scalar.activation(dst, po[:], AF.Copy)
                eng = nc.scalar if (split_stg and st == 15) else nc.sync
                eng.dma_start(outp[128 * st:128 * (st + 1), :], stg[:])

            # serial prefix: only what (blk0, hp0) needs — proj_b st0-3 and
            # the two qk chunks for head-pair 0, upper halves first so the
            # first scores (jts 7/6) start as soon as possible.
            for st in range(4):
                proj_b_chunk(st)
            proj_a_chunk(0, 0)
            proj_a_chunk(4, 0)

            def attention_iter(hp, blk, fillers):
                # jt3 (full-width) leads the score order for blk>0: its exp
                # and band-mult then sit at the HEAD of the ACT/DVE queues,
                # so the first PV (which must be jt3 — full-column PSUM
                # init) is never blocked behind the previous iteration's
                # normalize chain on DVE
                jts = [3, 0, 1, 2, 4, 5, 6, 7] if blk > 0 else [4, 5, 6, 7]
                first_jt = 3 if blk > 0 else 4
                pv_order = [first_jt] + [j for j in jts if j != first_jt]
                # paired psum: cols [0:512) head 2hp, [512:1024) head 2hp+1
                # rows 0-63: o numerator, rows 64-127: denominator replicas
                Op = aps.tile([128, 1024], F32, name=f"O{hp}_{blk}",
                              tag="Opair", bufs=1)
                Pt = {}
                # 'p' (held split-K out-proj opens) must allocate their PSUM
                # ring slots after ALL of this iter's S allocations, so they
                # only emit after the score loop
                inline_iter = iter([f for f in fillers if f[0] != 'p'])
                deferred = [f for f in fillers if f[0] == 'p']

                def emit(f):
                    if f is None:
                        return
                    kind, arg = f
                    if kind == 'a':
                        proj_a_chunk(*arg)
                    elif kind == 'b':
                        proj_b_chunk(arg)
                    elif kind == 'bd':
                        proj_b_chunk(arg, dve_copy=True)
                    elif kind in ('p', 'q'):
                        outproj_open(*arg)
                    elif kind == 'os':
                        outproj_chunk(arg, split_stg=True)
                    else:
                        outproj_chunk(arg)

                def emit_next_filler(tail=False):
                    emit(next(inline_iter, None))
                    if tail:
                        for f in deferred:
                            emit(f)
                        deferred.clear()

                for gi in range(0, len(jts), 2):
                    for jt in jts[gi:gi + 2]:
                        q0, w = _qrange(jt)
                        gsb = (blk - 1) * 512 + 128 * jt
                        Sp = aps.tile([128, 1024], F32,
                                      name=f"S{hp}_{blk}_{jt}", tag="S")
                        for par in (0, 1):
                            nc.tensor.matmul(
                                Sp[:, 512 * par:512 * par + w],
                                qkT[4 + hp][64 * par:64 * par + 64,
                                            gsb:gsb + 128],
                                qkT[hp][64 * par:64 * par + 64,
                                        512 * blk + q0:512 * blk + q0 + w],
                                start=True, stop=True,
                                tile_position=(64 * par, 0),
                                skip_group_check=True)
                        P = Ppool.tile([128, 1024], BF16,
                                       name=f"P{hp}_{blk}_{jt}", tag="P")
                        c0 = q0 - 128 * jt + 896
                        Pv = P.rearrange("p (two c) -> p two c", two=2)
                        Sv = Sp.rearrange("p (two c) -> p two c", two=2)
                        if blk > 0:
                            # paired exp + bias-mul (one op for both heads);
                            # route some band-mults to the idle Pool engine
                            nc.scalar.activation(Pv[:, :, 0:w], Sv[:, :, 0:w],
                                                 AF.Exp)
                            EBv = EB[hp].rearrange("p (two c) -> p two c",
                                                   two=2)
                            # late-PV jts' band-mults go to the idle Pool
                            # engine to unload DVE (their PVs run last)
                            meng = nc.gpsimd if jt >= 5 else nc.vector
                            meng.tensor_tensor(
                                Pv[:, :, 0:w], Pv[:, :, 0:w],
                                EBv[:, :, c0:c0 + w], ALU.mult)
                        else:
                            for par in (0, 1):
                                idx = (2 * hp + par) * 4 + (jt - 4)
                                nc.scalar.activation(
                                    P[:, 512 * par:512 * par + w],
                                    Sp[:, 512 * par:512 * par + w], AF.Exp,
                                    bias=B0[:, idx:idx + 1])
                            EMv = EM.rearrange("p (two c) -> p two c", two=2)
                            meng = nc.gpsimd if jt >= 6 else nc.vector
                            meng.tensor_tensor(
                                Pv[:, :, 0:w], Pv[:, :, 0:w],
                                EMv[:, :, 0:w], ALU.mult)
                        Pt[jt] = (P, q0, w)
                    # PE filler while ACT/DVE chew on the exps/muls
                    emit_next_filler()
                if blk == 0:
                    # lazy EB band load: sits behind this iter's Pool mults
                    # in gpsimd program order, so it starts only after the
                    # prefix-critical input DMAs have drained; first use is
                    # (blk1, same hp), several iterations later
                    nc.gpsimd.dma_start(EBv4[:, hp, :], ebig[:, hp, :])
                for i in range(4):
                    emit_next_filler()
                for i, jt in enumerate(pv_order):
                    st = 4 * (blk - 1) + jt
                    for par in (0, 1):
                        P, q0, w = Pt[jt]
                        hl = 2 * hp + par
                        nc.tensor.matmul(
                            Op[:, 512 * par + q0:512 * par + q0 + w],
                            VA[st][:, 128 * hl:128 * hl + 128],
                            P[:, 512 * par:512 * par + w],
                            start=(i == 0), stop=(i == len(pv_order) - 1),
                            skip_group_check=True)
                # held split-K opens go here: their PSUM ring slots only
                # free after this iter's last exps, and the PE would other-
                # wise idle while the normalize chain below runs
                for f in deferred:
                    emit(f)
                deferred.clear()
                # normalize: rows 64:128 hold the denominator replicated;
                # bounce to SBUF (approx_fast can't read PSUM accumulator
                # bits), reciprocal, then one mult per half writes f16 OT
                dnf = pa.tile([64, 1024], F32, name=f"dn{hp}_{blk}",
                              tag="dnf", bufs=2)
                rcp = pa.tile([64, 1024], F32, name=f"rc{hp}_{blk}",
                              tag="rcp", bufs=2)
                ot = OT[4 * hp + blk]
                if blk == 3 and hp == 3:
                    # last iteration: normalize by q-column halves in
                    # close-need order — cols 256:512 (both heads) feed
                    # close14 (qq=2) and chunk15 (qq=3) first, so the tail
                    # closes start while cols 0:256 still normalize
                    Opv = Op.rearrange("p (two c) -> p two c", two=2)
                    dnv = dnf.rearrange("p (two c) -> p two c", two=2)
                    rcv = rcp.rearrange("p (two c) -> p two c", two=2)
                    for ci, c0 in enumerate((256, 0)):
                        src = Opv[64:128, :, c0:c0 + 256]
                        if ci == 0:
                            nc.scalar.activation(dnv[:, :, c0:c0 + 256],
                                                 src, AF.Copy)
                        else:
                            nc.vector.tensor_copy(dnv[:, :, c0:c0 + 256],
                                                  src)
                        nc.vector.reciprocal_approx_fast(
                            rcv[:, :, c0:c0 + 256], dnv[:, :, c0:c0 + 256])
                        for par in (0, 1):
                            nc.vector.tensor_tensor(
                                ot[64 * par:64 * par + 64, c0:c0 + 256],
                                Op[0:64,
                                   512 * par + c0:512 * par + c0 + 256],
                                rcp[0:64,
                                    512 * par + c0:512 * par + c0 + 256],
                                ALU.mult)
                else:
                    # on DVE, not ACT: an ACT-side copy here lands ahead of
                    # the NEXT iteration's exps and stalls its score pipeline
                    nc.vector.tensor_copy(dnf[:], Op[64:128, :])
                    nc.vector.reciprocal_approx_fast(rcp[:], dnf[:])
                    for par in (0, 1):
                        nc.vector.tensor_tensor(
                            ot[64 * par:64 * par + 64, :],
                            Op[0:64, 512 * par:512 * par + 512],
                            rcp[0:64, 512 * par:512 * par + 512], ALU.mult)

            # blk-outer order: each iteration's fillers produce exactly the
            # qk/v chunks the NEXT iteration's scores need (just-in-time),
            # plus out-proj of completed blocks
            filler_plan = {
                # blk0 carries no b-chunks: they'd stall on the late
                # xT[:,512:1024] DMA; (1,0) needs them only by its pv
                (0, 0): [('a', (1, 0)), ('a', (5, 0))],
                (0, 1): [('a', (2, 0)), ('a', (6, 0))],
                (0, 2): [('a', (3, 0)), ('a', (7, 0))],
                (0, 3): [('a', (0, 1)), ('a', (4, 1))],
                (1, 0): [('a', (1, 1)), ('a', (5, 1)), ('bd', 4), ('bd', 5),
                         ('bd', 6), ('bd', 7)],
                (1, 1): [('a', (2, 1)), ('a', (6, 1)), ('bd', 8), ('o', 0)],
                (1, 2): [('a', (3, 1)), ('a', (7, 1)), ('bd', 9), ('o', 1)],
                (1, 3): [('a', (0, 2)), ('a', (4, 2)), ('bd', 10), ('o', 2)],
                (2, 0): [('a', (1, 2)), ('a', (5, 2)), ('bd', 11), ('o', 3)],
                (2, 1): [('a', (2, 2)), ('a', (6, 2)), ('bd', 12), ('o', 4)],
                (2, 2): [('a', (3, 2)), ('a', (7, 2)), ('bd', 13), ('o', 5)],
                (2, 3): [('a', (0, 3)), ('a', (4, 3)), ('bd', 14), ('bd', 15)],
                (3, 0): [('a', (1, 3)), ('a', (5, 3)), ('o', 6), ('o', 7)],
                (3, 1): [('a', (2, 3)), ('a', (6, 3)), ('o', 8)],
                (3, 2): [('a', (3, 3)), ('a', (7, 3)), ('o', 9)],
                (3, 3): [('os', 10), ('os', 11),
                         ('q', (14, (0, 1, 2), 'projch')),
                         ('p', (12, (0, 1, 2), 'S')),
                         ('p', (13, (0, 1, 2), 'S'))],
            }
            for blk in range(4):
                for hp in range(4):
                    attention_iter(hp, blk, filler_plan[(blk, hp)])
            # close 14 first (its projch slots gate st15's accumulator),
            # then chunk15: both need only the upper-half normalize; the
            # 12/13 closes wait for the lower half and overlap 15's cast
            outproj_close(14, (3,))
            outproj_chunk(15, split_stg=True)
            outproj_close(12, (3,))
            outproj_close(13, (3,))

    nc.compile()
    return nc


_NC = None


def _get_nc():
    global _NC
    if _NC is None:
        _NC = _build_nc()
    return _NC


def _host_consts():
    slopes = np.exp2(-(np.arange(H, dtype=np.float64) + 1.0) * 8.0 / H)
    p = np.arange(128)[:, None]
    c = np.arange(1408)[None, :]
    delta = (c - p - 384).astype(np.float64)
    valid = (delta >= 0) & (delta <= 512)
    eb = np.zeros((H, 128, 1408), ml_dtypes.bfloat16)
    for h in range(H):
        vals = np.exp(slopes[h] * (delta - 512.0) - CM)
        eb[h] = np.where(valid, vals, 0.0).astype(ml_dtypes.bfloat16)
    cc = np.arange(512)[None, :]
    em0 = (cc >= p).astype(ml_dtypes.bfloat16)
    em0 = np.concatenate([em0, em0], axis=1)  # paired [128, 1024]
    # pair-interleaved bands: [g, hp, 128, 2*1408]
    ebp = np.zeros((2, 4, 128, 2816), ml_dtypes.bfloat16)
    for g in range(2):
        for hp in range(4):
            ebp[g, hp, :, 0:1408] = eb[8 * g + 2 * hp]
            ebp[g, hp, :, 1408:2816] = eb[8 * g + 2 * hp + 1]
    b0 = np.zeros((2, 128, 32), np.float32)  # per head-group
    for g in range(2):
        for hl in range(HPC):
            for jtl in range(4):
                b0[g, :, hl * 4 + jtl] = (
                    -slopes[8 * g + hl] * (128.0 * jtl + p[:, 0]) - CM)
    return slopes, ebp, em0, b0


def kernel(x, w_in, w_out):
    global LAST_RESULTS
    x = np.asarray(x, dtype=np.float32)
    w_in = np.asarray(w_in, dtype=np.float32)
    w_out = np.asarray(w_out, dtype=np.float32)

    nc = _get_nc()
    _, ebp, em0, b0 = _host_consts()

    def pack(a, nk):
        # [128*nk, C] -> [128, nk, C] (SBUF big-tile layout, one DMA each)
        return np.ascontiguousarray(
            a.reshape(nk, 128, a.shape[1]).transpose(1, 0, 2))

    in_maps = []
    for core in range(NCORES):
        b, g = divmod(core, 2)
        r0 = 512 * g
        w_qk = np.concatenate(
            [w_in[r0:r0 + 512] * 0.125,
             w_in[E + r0:E + r0 + 512]], axis=0).T.astype(np.float16)
        w_v = w_in[2 * E + r0:2 * E + r0 + 512].T.astype(np.float16)
        w_o = w_out[:, r0:r0 + 512].T.astype(np.float16)
        xTc = x[b].T.astype(np.float16)
        # [1024, 2048] -> [p, colblock, k, 512]
        xTp = np.ascontiguousarray(
            xTc.reshape(8, 128, 4, 512).transpose(1, 2, 0, 3))
        in_maps.append({
            "xT": xTp,
            "w_qk": pack(w_qk, 8),
            "w_v": pack(w_v, 8),
            "w_o": pack(w_o, 4),
            "expbig": np.ascontiguousarray(ebp[g].transpose(1, 0, 2)),
            "em0": em0,
            "b0v": np.ascontiguousarray(b0[g]),
        })

    res = run_bass_kernel_spmd(nc, in_maps, core_ids=list(range(NCORES)))
    LAST_RESULTS = res
    out = np.stack([
        res.results[2 * b]["out_p"].astype(np.float32)
        + res.results[2 * b + 1]["out_p"].astype(np.float32)
        for b in range(B)
    ])
    return out



# revision 72
# speedup vs baseline: 1.1996x; 1.1996x over previous
"""Trainium2 Bass kernel: sliding-window multihead attention w/ ALiBi.

Computation (per reference):
  qkv = x @ w_in.T ; q,k,v heads ; blocked sliding-window causal attention
  (window=512, ALiBi bias slope_h*(q_idx-kv_idx)) ; out = o @ w_out.T

Sharding: 8 cores = 4 batches x 2 head-groups (8 heads each). Each core
computes its batch's QKV for its heads, attention, and a partial out-proj
over its heads' columns. Host sums the two head-group partials per batch.

Softmax trick: P = exp(s_raw) * EXPBIG where EXPBIG = exp(bias - bound)
is a host-precomputed Toeplitz band (exact 0 outside the valid window).
The row-max subtraction is replaced by a static bound folded into EXPBIG
(block 0 uses a per-partition ACT bias instead). Each head's PV stationary
is [v (64 cols) | ones (64 cols)], so the PV matmul lands the softmax
denominator replicated across PSUM rows 64:128 and a single DVE divide
per half produces the normalized output tile.
"""

import os
import numpy as np
import ml_dtypes
from contextlib import ExitStack

import concourse.bass as bass
import concourse.bacc as bacc
import concourse.tile as tile
import concourse.mybir as mybir
from concourse.bass_utils import run_bass_kernel_spmd

F16 = mybir.dt.float16
BF16 = mybir.dt.bfloat16
F32 = mybir.dt.float32
AF = mybir.ActivationFunctionType
ALU = mybir.AluOpType

B, S, E = 4, 2048, 1024
H, D, WIN = 16, 64, 512
NB = S // WIN          # 4 blocks
HPC = 8                # heads per core
NCORES = 8
CM = 6.0               # softmax bound safety margin

LAST_RESULTS = None


def _qrange(jt):
    # valid q-column range for scores j-tile jt (window band)
    lo = max(0, 128 * jt - 512)
    hi = min(512, 128 * jt + 128)
    return lo, hi - lo


def _build_nc():
    nc = bacc.Bacc("TRN2", target_bir_lowering=False, debug=False,
                   num_devices=NCORES)

    # host pre-packs every tensor in its SBUF layout ([128, k, cols]) so
    # each one loads with a single large DMA: per-queue dma_start issue
    # overhead (~0.9us each) was the real input-phase bottleneck
    # xT packed [p, colblock, k, 512]: per-partition lines are 8KB contiguous
    # (k-major inside a column block), so each col-block loads as one fast DMA
    xT = nc.dram_tensor("xT", [128, 4, 8, 512], F16,
                        kind="ExternalInput").ap()
    wqk = nc.dram_tensor("w_qk", [128, 8, 1024], F16,
                         kind="ExternalInput").ap()
    wv = nc.dram_tensor("w_v", [128, 8, 512], F16, kind="ExternalInput").ap()
    wo = nc.dram_tensor("w_o", [128, 4, 1024], F16,
                        kind="ExternalInput").ap()
    ebig = nc.dram_tensor("expbig", [128, 4, 2816], BF16,
                          kind="ExternalInput").ap()
    em0 = nc.dram_tensor("em0", [128, 1024], BF16, kind="ExternalInput").ap()
    b0v = nc.dram_tensor("b0v", [128, 32], F32, kind="ExternalInput").ap()
    outp = nc.dram_tensor("out_p", [S, E], F16, kind="ExternalOutput").ap()

    with tile.TileContext(nc) as tc, ExitStack() as ctx:
        pp = ctx.enter_context(tc.tile_pool(name="persist", bufs=1))

        # persistent SBUF tensors
        qkT = [pp.tile([128, S], F16, name=f"qkT{m}", tag=f"qkT{m}")
               for m in range(8)]                       # f-major qk.T
        VA = [pp.tile([128, HPC * 128], BF16, name=f"VA{s}", tag=f"VA{s}")
              for s in range(16)]                  # per head: v(64) | ones(64)
        OT = [pp.tile([128, 512], F16, name=f"OT{i}", tag=f"OT{i}")
              for i in range(16)]                       # normalized o.T
        EBB = pp.tile([128, 4 * 2816], BF16, name="EBB", tag="EBB")
        EBv4 = EBB.rearrange("p (h c) -> p h c", h=4)
        EB = [EBv4[:, h, :] for h in range(4)]     # exp(bias-bound) band pairs
        EM = pp.tile([128, 1024], BF16, name="EM", tag="EM")  # blk0 causal 0/1
        B0 = pp.tile([128, 32], F32, name="B0", tag="B0")    # blk0 exp biases
        WOB = pp.tile([128, 4 * 1024], F16, name="WOB", tag="WOB")
        WOv = WOB.rearrange("p (k c) -> p k c", k=4)
        WO = [WOv[:, k, :] for k in range(4)]

        with tc.tile_pool(name="phA", bufs=1) as pa, \
             tc.tile_pool(name="Pp", bufs=8) as Ppool, \
             tc.tile_pool(name="aps", bufs=2, space="PSUM") as aps:
            XTB = pa.tile([128, 8 * S], F16, name="XTB", tag="XTB")
            XTv = XTB.rearrange("p (cb k r) -> p cb k r", cb=4, k=8)

            def xts(kt, c0, w):
                # xT column range [c0, c0+w) of chunk kt; must stay inside
                # one 512-col block
                cb, r = divmod(c0, 512)
                return XTv[:, cb, kt, r:r + w]
            WQKB = pa.tile([128, 8 * 1024], F16, name="WQKB", tag="WQKB")
            WQKv = WQKB.rearrange("p (k c) -> p k c", k=8)
            wqks = [WQKv[:, k, :] for k in range(8)]
            WVB = pa.tile([128, 8 * 512], F16, name="WVB", tag="WVB")
            WVv = WVB.rearrange("p (k c) -> p k c", k=8)
            wvs = [WVv[:, k, :] for k in range(8)]
            wrm = pa.tile([128, 256], F16, name="wrm", tag="wrm")

            # warm-up weights first so the PE can start ramping immediately
            nc.gpsimd.memset(wrm[:], 0.0)

            # -- input DMA: one large transfer per tensor / 512-col xT slice
            # (per-dma_start issue overhead ~0.9us was the input bottleneck;
            # host pre-packs so per-partition lines are >=8KB contiguous).
            # Single prioritized FIFO on the sync queue in strict need-order:
            # one queue alone sustains ~400GB/s, and parallel queues would
            # fair-share the bus and starve the prefix-critical transfers.
            nc.sync.dma_start(WVv[:], wv[:])
            nc.sync.dma_start(XTv[:, 0], xT[:, 0])
            nc.sync.dma_start(WQKv[:], wqk[:])
            nc.sync.dma_start(XTv[:, 1], xT[:, 1])
            nc.sync.dma_start(XTv[:, 2], xT[:, 2])
            nc.sync.dma_start(XTv[:, 3], xT[:, 3])
            nc.sync.dma_start(WOv[:], wo[:])
            nc.scalar.dma_start(B0[:], b0v[:])
            # EM now; EB bands are issued lazily inside the blk0 iterations
            # (one per head-pair) so they never starve the prefix transfers
            nc.gpsimd.dma_start(EM[:], em0[:])

            # HAM warm-up: dummy matmuls ramp the PE p-state while DMAs land
            wps = aps.tile([128, 512], F32, name="wps", tag="projch", bufs=2)
            NWARM = 16
            for i in range(NWARM):
                nc.tensor.matmul(wps[:, 0:128], wrm[:, 0:128],
                                 wrm[:, 0:128],
                                 start=(i == 0), stop=(i == NWARM - 1))
            # DMA-paced warm-up: dummy matmuls gated on arriving wv chunks
            # keep a low-duty PE heartbeat through the DMA-fill window, so
            # the activity manager grants full clock before the projection
            # phase without the burst-then-claw-back pattern
            wps2 = aps.tile([128, 512], F32, name="wps2", tag="projch",
                            bufs=2)
            for k in range(8):
                nc.tensor.matmul(wps2[:], wvs[k][:, 0:128],
                                 wvs[k][:],
                                 start=(k == 0), stop=(k == 7))

            # ones columns for the denominator replicas (v halves get
            # overwritten by proj_b)
            for st in range(16):
                ones_v = VA[st].rearrange("p (h c) -> p h c", h=HPC)
                nc.gpsimd.memset(ones_v[:, :, 64:128], 1.0)

            # ---- projection b chunk: v[s, f] into VA v-halves -------------
            def proj_b_chunk(st, dve_copy=False):
                pv = aps.tile([128, 512], F32, name=f"pv{st}", tag="projch",
                              bufs=2)
                for kt in range(8):
                    nc.tensor.matmul(
                        pv[:],
                        xts(kt, 128 * st, 128),
                        wvs[kt][:],
                        start=(kt == 0), stop=(kt == 7))
                src = pv.rearrange("p (h c) -> p h c", h=HPC)
                dst = VA[st].rearrange("p (h c) -> p h c", h=HPC)
                if dve_copy:
                    nc.vector.tensor_copy(dst[:, :, 0:64], src[:])
                else:
                    nc.scalar.activation(dst[:, :, 0:64], src[:], AF.Copy)

            # ---- projection a: qkT[f, s], one (mt, sc) chunk at a time ----
            def proj_a_chunk(mt, sc):
                ps = aps.tile([128, 512], F32, name=f"pa{mt}_{sc}",
                              tag="projch", bufs=2)
                for kt in range(8):
                    nc.tensor.matmul(
                        ps[:],
                        wqks[kt][:, 128 * mt:128 * (mt + 1)],
                        xts(kt, 512 * sc, 512),
                        start=(kt == 0), stop=(kt == 7))
                # split the PSUM->SBUF casts across ACT and DVE to balance
                # engine load (GPSIMD cannot read PSUM)
                if (mt + sc) % 2 == 0:
                    nc.scalar.activation(qkT[mt][:, 512 * sc:512 * (sc + 1)],
                                         ps[:], AF.Copy)
                else:
                    nc.vector.tensor_copy(qkT[mt][:, 512 * sc:512 * (sc + 1)],
                                          ps[:])

            # ---- split-K out-projection: open partial sums over a subset of
            # head-pairs (their OT blocks are ready early), close with the
            # rest once the final head-pair lands ----
            po_hold = {}

            def outproj_open(st, kts, tag):
                blk_, qq = st // 4, st % 4
                if tag == "S":
                    po = aps.tile([128, 1024], F32, name=f"poH{st}", tag="S")
                    halves = [po[:, 0:512], po[:, 512:1024]]
                else:
                    halves = [aps.tile([128, 512], F32, name=f"poH{st}_{h}",
                                       tag="projch", bufs=2)[:]
                              for h in range(2)]
                po_hold[st] = halves
                for half in range(2):
                    for i, kt in enumerate(kts):
                        nc.tensor.matmul(
                            halves[half],
                            OT[4 * kt + blk_][:, 128 * qq:128 * (qq + 1)],
                            WO[kt][:, 512 * half:512 * half + 512],
                            start=(i == 0), stop=False,
                            skip_group_check=True)

            def outproj_close(st, kts):
                blk_, qq = st // 4, st % 4
                halves = po_hold[st]
                stg = pa.tile([128, 1024], F16, name=f"stgH{st}",
                              tag="stg", bufs=3)
                for half in range(2):
                    for i, kt in enumerate(kts):
                        nc.tensor.matmul(
                            halves[half],
                            OT[4 * kt + blk_][:, 128 * qq:128 * (qq + 1)],
                            WO[kt][:, 512 * half:512 * half + 512],
                            start=False, stop=(i == len(kts) - 1),
                            skip_group_check=True)
                    # tail: split the PSUM->SBUF casts across ACT and DVE
                    # so they drain in parallel after the last matmul
                    dst = stg[:, 512 * half:512 * half + 512]
                    if half == 0:
                        nc.scalar.activation(dst, halves[half], AF.Copy)
                    else:
                        nc.vector.tensor_copy(dst, halves[half])
                # one full-row 2KB-line DMA; strictly alternate queues in
                # emission order (14,12,13 -> sync,scalar,sync) so the tail
                # drains evenly on both
                eng = nc.scalar if st == 12 else nc.sync
                eng.dma_start(outp[128 * st:128 * (st + 1), :], stg[:])

            # ---- out-projection chunk (one s-tile) ----
            def outproj_chunk(st, split_stg=False):
                blk_, qq = st // 4, st % 4
                # both halves stage into one [128, 2KB] tile -> a single
                # full-row DMA (2KB lines = half the packets of 1KB lines)
                stg = pa.tile([128, 1024], F16, name=f"stg{st}",
                              tag="stg", bufs=3)
                for half in range(2):
                    po = aps.tile([128, 512], F32, name=f"po{st}_{half}",
                                  tag="projch", bufs=2)
                    for kt in range(4):
                        nc.tensor.matmul(
                            po[:],
                            OT[4 * kt + blk_][:, 128 * qq:128 * (qq + 1)],
                            WO[kt][:, 512 * half:512 * (half + 1)],
                            start=(kt == 0), stop=(kt == 3))
                    dst = stg[:, 512 * half:512 * (half + 1)]
                    if split_stg and half == 1:
                        nc.vector.tensor_copy(dst, po[:])
                    else:
                        nc.scalar.activation(dst, po[:], AF.Copy)
                eng = nc.scalar if (split_stg and st == 15) else nc.sync
                eng.dma_start(outp[128 * st:128 * (st + 1), :], stg[:])

            # serial prefix: only what (blk0, hp0) needs — proj_b st0-3 and
            # the two qk chunks for head-pair 0, upper halves first so the
            # first scores (jts 7/6) start as soon as possible.
            for st in range(4):
                proj_b_chunk(st)
            proj_a_chunk(0, 0)
            proj_a_chunk(4, 0)

            def attention_iter(hp, blk, fillers):
                jts = list(range(8)) if blk > 0 else [4, 5, 6, 7]
                first_jt = 3 if blk > 0 else 4
                pv_order = [first_jt] + [j for j in jts if j != first_jt]
                # paired psum: cols [0:512) head 2hp, [512:1024) head 2hp+1
                # rows 0-63: o numerator, rows 64-127: denominator replicas
                Op = aps.tile([128, 1024], F32, name=f"O{hp}_{blk}",
                              tag="Opair", bufs=1)
                Pt = {}
                # 'p' (held split-K out-proj opens) must allocate their PSUM
                # ring slots after ALL of this iter's S allocations, so they
                # only emit after the score loop
                inline_iter = iter([f for f in fillers if f[0] != 'p'])
                deferred = [f for f in fillers if f[0] == 'p']

                def emit(f):
                    if f is None:
                        return
                    kind, arg = f
                    if kind == 'a':
                        proj_a_chunk(*arg)
                    elif kind == 'b':
                        proj_b_chunk(arg)
                    elif kind == 'bd':
                        proj_b_chunk(arg, dve_copy=True)
                    elif kind in ('p', 'q'):
                        outproj_open(*arg)
                    elif kind == 'os':
                        outproj_chunk(arg, split_stg=True)
                    else:
                        outproj_chunk(arg)

                def emit_next_filler(tail=False):
                    emit(next(inline_iter, None))
                    if tail:
                        for f in deferred:
                            emit(f)
                        deferred.clear()

                for gi in range(0, len(jts), 2):
                    for jt in jts[gi:gi + 2]:
                        q0, w = _qrange(jt)
                        gsb = (blk - 1) * 512 + 128 * jt
                        Sp = aps.tile([128, 1024], F32,
                                      name=f"S{hp}_{blk}_{jt}", tag="S")
                        for par in (0, 1):
                            nc.tensor.matmul(
                                Sp[:, 512 * par:512 * par + w],
                                qkT[4 + hp][64 * par:64 * par + 64,
                                            gsb:gsb + 128],
                                qkT[hp][64 * par:64 * par + 64,
                                        512 * blk + q0:512 * blk + q0 + w],
                                start=True, stop=True,
                                tile_position=(64 * par, 0),
                                skip_group_check=True)
                        P = Ppool.tile([128, 1024], BF16,
                                       name=f"P{hp}_{blk}_{jt}", tag="P")
                        c0 = q0 - 128 * jt + 896
                        Pv = P.rearrange("p (two c) -> p two c", two=2)
                        Sv = Sp.rearrange("p (two c) -> p two c", two=2)
                        if blk > 0:
                            # paired exp + bias-mul (one op for both heads);
                            # route some band-mults to the idle Pool engine
                            nc.scalar.activation(Pv[:, :, 0:w], Sv[:, :, 0:w],
                                                 AF.Exp)
                            EBv = EB[hp].rearrange("p (two c) -> p two c",
                                                   two=2)
                            # late-PV jts' band-mults go to the idle Pool
                            # engine to unload DVE (their PVs run last)
                            meng = nc.gpsimd if jt >= 5 else nc.vector
                            meng.tensor_tensor(
                                Pv[:, :, 0:w], Pv[:, :, 0:w],
                                EBv[:, :, c0:c0 + w], ALU.mult)
                        else:
                            for par in (0, 1):
                                idx = (2 * hp + par) * 4 + (jt - 4)
                                nc.scalar.activation(
                                    P[:, 512 * par:512 * par + w],
                                    Sp[:, 512 * par:512 * par + w], AF.Exp,
                                    bias=B0[:, idx:idx + 1])
                            EMv = EM.rearrange("p (two c) -> p two c", two=2)
                            meng = nc.gpsimd if jt >= 6 else nc.vector
                            meng.tensor_tensor(
                                Pv[:, :, 0:w], Pv[:, :, 0:w],
                                EMv[:, :, 0:w], ALU.mult)
                        Pt[jt] = (P, q0, w)
                    # PE filler while ACT/DVE chew on the exps/muls
                    emit_next_filler()
                if blk == 0:
                    # lazy EB band load: sits behind this iter's Pool mults
                    # in gpsimd program order, so it starts only after the
                    # prefix-critical input DMAs have drained; first use is
                    # (blk1, same hp), several iterations later
                    nc.gpsimd.dma_start(EBv4[:, hp, :], ebig[:, hp, :])
                for i in range(4):
                    emit_next_filler()
                for i, jt in enumerate(pv_order):
                    st = 4 * (blk - 1) + jt
                    for par in (0, 1):
                        P, q0, w = Pt[jt]
                        hl = 2 * hp + par
                        nc.tensor.matmul(
                            Op[:, 512 * par + q0:512 * par + q0 + w],
                            VA[st][:, 128 * hl:128 * hl + 128],
                            P[:, 512 * par:512 * par + w],
                            start=(i == 0), stop=(i == len(pv_order) - 1),
                            skip_group_check=True)
                # held split-K opens go here: their PSUM ring slots only
                # free after this iter's last exps, and the PE would other-
                # wise idle while the normalize chain below runs
                for f in deferred:
                    emit(f)
                deferred.clear()
                # normalize: rows 64:128 hold the denominator replicated;
                # bounce to SBUF (approx_fast can't read PSUM accumulator
                # bits), reciprocal, then one mult per half writes f16 OT
                dnf = pa.tile([64, 1024], F32, name=f"dn{hp}_{blk}",
                              tag="dnf", bufs=2)
                rcp = pa.tile([64, 1024], F32, name=f"rc{hp}_{blk}",
                              tag="rcp", bufs=2)
                ot = OT[4 * hp + blk]
                if blk == 3 and hp == 3:
                    # last iteration: normalize by q-column halves in
                    # close-need order — cols 256:512 (both heads) feed
                    # close14 (qq=2) and chunk15 (qq=3) first, so the tail
                    # closes start while cols 0:256 still normalize
                    Opv = Op.rearrange("p (two c) -> p two c", two=2)
                    dnv = dnf.rearrange("p (two c) -> p two c", two=2)
                    rcv = rcp.rearrange("p (two c) -> p two c", two=2)
                    for ci, c0 in enumerate((256, 0)):
                        src = Opv[64:128, :, c0:c0 + 256]
                        if ci == 0:
                            nc.scalar.activation(dnv[:, :, c0:c0 + 256],
                                                 src, AF.Copy)
                        else:
                            nc.vector.tensor_copy(dnv[:, :, c0:c0 + 256],
                                                  src)
                        nc.vector.reciprocal_approx_fast(
                            rcv[:, :, c0:c0 + 256], dnv[:, :, c0:c0 + 256])
                        for par in (0, 1):
                            nc.vector.tensor_tensor(
                                ot[64 * par:64 * par + 64, c0:c0 + 256],
                                Op[0:64,
                                   512 * par + c0:512 * par + c0 + 256],
                                rcp[0:64,
                                    512 * par + c0:512 * par + c0 + 256],
                                ALU.mult)
                else:
                    # on DVE, not ACT: an ACT-side copy here lands ahead of
                    # the NEXT iteration's exps and stalls its score pipeline
                    nc.vector.tensor_copy(dnf[:], Op[64:128, :])
                    nc.vector.reciprocal_approx_fast(rcp[:], dnf[:])
                    for par in (0, 1):
                        nc.vector.tensor_tensor(
                            ot[64 * par:64 * par + 64, :],
                            Op[0:64, 512 * par:512 * par + 512],
                            rcp[0:64, 512 * par:512 * par + 512], ALU.mult)

            # blk-outer order: each iteration's fillers produce exactly the
            # qk/v chunks the NEXT iteration's scores need (just-in-time),
            # plus out-proj of completed blocks
            filler_plan = {
                # blk0 carries no b-chunks: they'd stall on the late
                # xT[:,512:1024] DMA; (1,0) needs them only by its pv
                (0, 0): [('a', (1, 0)), ('a', (5, 0))],
                (0, 1): [('a', (2, 0)), ('a', (6, 0))],
                (0, 2): [('a', (3, 0)), ('a', (7, 0))],
                (0, 3): [('a', (0, 1)), ('a', (4, 1))],
                (1, 0): [('a', (1, 1)), ('a', (5, 1)), ('bd', 4), ('bd', 5),
                         ('bd', 6), ('bd', 7)],
                (1, 1): [('a', (2, 1)), ('a', (6, 1)), ('bd', 8), ('o', 0)],
                (1, 2): [('a', (3, 1)), ('a', (7, 1)), ('bd', 9), ('o', 1)],
                (1, 3): [('a', (0, 2)), ('a', (4, 2)), ('bd', 10), ('o', 2)],
                (2, 0): [('a', (1, 2)), ('a', (5, 2)), ('bd', 11), ('o', 3)],
                (2, 1): [('a', (2, 2)), ('a', (6, 2)), ('bd', 12), ('o', 4)],
                (2, 2): [('a', (3, 2)), ('a', (7, 2)), ('bd', 13), ('o', 5)],
                (2, 3): [('a', (0, 3)), ('a', (4, 3)), ('bd', 14), ('bd', 15)],
                (3, 0): [('a', (1, 3)), ('a', (5, 3)), ('o', 6), ('o', 7)],
                (3, 1): [('a', (2, 3)), ('a', (6, 3)), ('o', 8)],
                (3, 2): [('a', (3, 3)), ('a', (7, 3)), ('o', 9)],
                (3, 3): [('os', 10), ('os', 11),
                         ('q', (14, (0, 1, 2), 'projch')),
                         ('p', (12, (0, 1, 2), 'S')),
                         ('p', (13, (0, 1, 2), 'S'))],
            }
            for blk in range(4):
                for hp in range(4):
                    attention_iter(hp, blk, filler_plan[(blk, hp)])
            # close 14 first (its projch slots gate st15's accumulator),
            # then chunk15: both need only the upper-half normalize; the
            # 12/13 closes wait for the lower half and overlap 15's cast
            outproj_close(14, (3,))
            outproj_chunk(15, split_stg=True)
            outproj_close(12, (3,))
            outproj_close(13, (3,))

    nc.compile()
    return nc


_NC = None


def _get_nc():
    global _NC
    if _NC is None:
        _NC = _build_nc()
    return _NC


def _host_consts():
    slopes = np.exp2(-(np.arange(H, dtype=np.float64) + 1.0) * 8.0 / H)
    p = np.arange(128)[:, None]
    c = np.arange(1408)[None, :]
    delta = (c - p - 384).astype(np.float64)
    valid = (delta >= 0) & (delta <= 512)
    eb = np.zeros((H, 128, 1408), ml_dtypes.bfloat16)
    for h in range(H):
        vals = np.exp(slopes[h] * (delta - 512.0) - CM)
        eb[h] = np.where(valid, vals, 0.0).astype(ml_dtypes.bfloat16)
    cc = np.arange(512)[None, :]
    em0 = (cc >= p).astype(ml_dtypes.bfloat16)
    em0 = np.concatenate([em0, em0], axis=1)  # paired [128, 1024]
    # pair-interleaved bands: [g, hp, 128, 2*1408]
    ebp = np.zeros((2, 4, 128, 2816), ml_dtypes.bfloat16)
    for g in range(2):
        for hp in range(4):
            ebp[g, hp, :, 0:1408] = eb[8 * g + 2 * hp]
            ebp[g, hp, :, 1408:2816] = eb[8 * g + 2 * hp + 1]
    b0 = np.zeros((2, 128, 32), np.float32)  # per head-group
    for g in range(2):
        for hl in range(HPC):
            for jtl in range(4):
                b0[g, :, hl * 4 + jtl] = (
                    -slopes[8 * g + hl] * (128.0 * jtl + p[:, 0]) - CM)
    return slopes, ebp, em0, b0


def kernel(x, w_in, w_out):
    global LAST_RESULTS
    x = np.asarray(x, dtype=np.float32)
    w_in = np.asarray(w_in, dtype=np.float32)
    w_out = np.asarray(w_out, dtype=np.float32)

    nc = _get_nc()
    _, ebp, em0, b0 = _host_consts()

    def pack(a, nk):
        # [128*nk, C] -> [128, nk, C] (SBUF big-tile layout, one DMA each)
        return np.ascontiguousarray(
            a.reshape(nk, 128, a.shape[1]).transpose(1, 0, 2))

    in_maps = []
    for core in range(NCORES):
        b, g = divmod(core, 2)
        r0 = 512 * g
        w_qk = np.concatenate(
            [w_in[r0:r0 + 512] * 0.125,
             w_in[E + r0:E + r0 + 512]], axis=0).T.astype(np.float16)
        w_v = w_in[2 * E + r0:2 * E + r0 + 512].T.astype(np.float16)
        w_o = w_out[:, r0:r0 + 512].T.astype(np.float16)
        xTc = x[b].T.astype(np.float16)
        # [1024, 2048] -> [p, colblock, k, 512]
        xTp = np.ascontiguousarray(
            xTc.reshape(8, 128, 4, 512).transpose(1, 2, 0, 3))
        in_maps.append({
            "xT": xTp,
            "w_qk": pack(w_qk, 8),
            "w_v": pack(w_v, 8),
            "w_o": pack(w_o, 4),
            "expbig": np.ascontiguousarray(ebp[g].transpose(1, 0, 2)),
            "em0": em0,
            "b0v": np.ascontiguousarray(b0[g]),
        })

    res = run_bass_kernel_spmd(nc, in_maps, core_ids=list(range(NCORES)))
    LAST_RESULTS = res
    out = np.stack([
        res.results[2 * b]["out_p"].astype(np.float32)
        + res.results[2 * b + 1]["out_p"].astype(np.float32)
        for b in range(B)
    ])
    return out



# revision 73
# speedup vs baseline: 1.2061x; 1.0055x over previous
"""Trainium2 Bass kernel: sliding-window multihead attention w/ ALiBi.

Computation (per reference):
  qkv = x @ w_in.T ; q,k,v heads ; blocked sliding-window causal attention
  (window=512, ALiBi bias slope_h*(q_idx-kv_idx)) ; out = o @ w_out.T

Sharding: 8 cores = 4 batches x 2 head-groups (8 heads each). Each core
computes its batch's QKV for its heads, attention, and a partial out-proj
over its heads' columns. Host sums the two head-group partials per batch.

Softmax trick: P = exp(s_raw) * EXPBIG where EXPBIG = exp(bias - bound)
is a host-precomputed Toeplitz band (exact 0 outside the valid window).
The row-max subtraction is replaced by a static bound folded into EXPBIG
(block 0 uses a per-partition ACT bias instead). Each head's PV stationary
is [v (64 cols) | ones (64 cols)], so the PV matmul lands the softmax
denominator replicated across PSUM rows 64:128 and a single DVE divide
per half produces the normalized output tile.
"""

import os
import numpy as np
import ml_dtypes
from contextlib import ExitStack

import concourse.bass as bass
import concourse.bacc as bacc
import concourse.tile as tile
import concourse.mybir as mybir
from concourse.bass_utils import run_bass_kernel_spmd

F16 = mybir.dt.float16
BF16 = mybir.dt.bfloat16
F32 = mybir.dt.float32
AF = mybir.ActivationFunctionType
ALU = mybir.AluOpType

B, S, E = 4, 2048, 1024
H, D, WIN = 16, 64, 512
NB = S // WIN          # 4 blocks
HPC = 8                # heads per core
NCORES = 8
CM = 6.0               # softmax bound safety margin

LAST_RESULTS = None


def _qrange(jt):
    # valid q-column range for scores j-tile jt (window band)
    lo = max(0, 128 * jt - 512)
    hi = min(512, 128 * jt + 128)
    return lo, hi - lo


def _build_nc():
    nc = bacc.Bacc("TRN2", target_bir_lowering=False, debug=False,
                   num_devices=NCORES)

    # host pre-packs every tensor in its SBUF layout ([128, k, cols]) so
    # each one loads with a single large DMA: per-queue dma_start issue
    # overhead (~0.9us each) was the real input-phase bottleneck
    # xT packed [p, colblock, k, 512]: per-partition lines are 8KB contiguous
    # (k-major inside a column block), so each col-block loads as one fast DMA
    xT = nc.dram_tensor("xT", [128, 4, 8, 512], F16,
                        kind="ExternalInput").ap()
    wqk = nc.dram_tensor("w_qk", [128, 8, 1024], F16,
                         kind="ExternalInput").ap()
    wv = nc.dram_tensor("w_v", [128, 8, 512], F16, kind="ExternalInput").ap()
    wo = nc.dram_tensor("w_o", [128, 4, 1024], F16,
                        kind="ExternalInput").ap()
    ebig = nc.dram_tensor("expbig", [128, 4, 2816], BF16,
                          kind="ExternalInput").ap()
    em0 = nc.dram_tensor("em0", [128, 1024], BF16, kind="ExternalInput").ap()
    b0v = nc.dram_tensor("b0v", [128, 32], F32, kind="ExternalInput").ap()
    outp = nc.dram_tensor("out_p", [S, E], F16, kind="ExternalOutput").ap()

    with tile.TileContext(nc) as tc, ExitStack() as ctx:
        pp = ctx.enter_context(tc.tile_pool(name="persist", bufs=1))

        # persistent SBUF tensors
        qkT = [pp.tile([128, S], F16, name=f"qkT{m}", tag=f"qkT{m}")
               for m in range(8)]                       # f-major qk.T
        VA = [pp.tile([128, HPC * 128], BF16, name=f"VA{s}", tag=f"VA{s}")
              for s in range(16)]                  # per head: v(64) | ones(64)
        OT = [pp.tile([128, 512], F16, name=f"OT{i}", tag=f"OT{i}")
              for i in range(16)]                       # normalized o.T
        EBB = pp.tile([128, 4 * 2816], BF16, name="EBB", tag="EBB")
        EBv4 = EBB.rearrange("p (h c) -> p h c", h=4)
        EB = [EBv4[:, h, :] for h in range(4)]     # exp(bias-bound) band pairs
        EM = pp.tile([128, 1024], BF16, name="EM", tag="EM")  # blk0 causal 0/1
        B0 = pp.tile([128, 32], F32, name="B0", tag="B0")    # blk0 exp biases
        WOB = pp.tile([128, 4 * 1024], F16, name="WOB", tag="WOB")
        WOv = WOB.rearrange("p (k c) -> p k c", k=4)
        WO = [WOv[:, k, :] for k in range(4)]

        with tc.tile_pool(name="phA", bufs=1) as pa, \
             tc.tile_pool(name="Pp", bufs=8) as Ppool, \
             tc.tile_pool(name="aps", bufs=2, space="PSUM") as aps:
            XTB = pa.tile([128, 8 * S], F16, name="XTB", tag="XTB")
            XTv = XTB.rearrange("p (cb k r) -> p cb k r", cb=4, k=8)

            def xts(kt, c0, w):
                # xT column range [c0, c0+w) of chunk kt; must stay inside
                # one 512-col block
                cb, r = divmod(c0, 512)
                return XTv[:, cb, kt, r:r + w]
            WQKB = pa.tile([128, 8 * 1024], F16, name="WQKB", tag="WQKB")
            WQKv = WQKB.rearrange("p (k c) -> p k c", k=8)
            wqks = [WQKv[:, k, :] for k in range(8)]
            WVB = pa.tile([128, 8 * 512], F16, name="WVB", tag="WVB")
            WVv = WVB.rearrange("p (k c) -> p k c", k=8)
            wvs = [WVv[:, k, :] for k in range(8)]
            wrm = pa.tile([128, 256], F16, name="wrm", tag="wrm")

            # warm-up weights first so the PE can start ramping immediately
            nc.gpsimd.memset(wrm[:], 0.0)

            # -- input DMA: one large transfer per tensor / 512-col xT slice
            # (per-dma_start issue overhead ~0.9us was the input bottleneck;
            # host pre-packs so per-partition lines are >=8KB contiguous).
            # Single prioritized FIFO on the sync queue in strict need-order:
            # one queue alone sustains ~400GB/s, and parallel queues would
            # fair-share the bus and starve the prefix-critical transfers.
            nc.sync.dma_start(WVv[:], wv[:])
            nc.sync.dma_start(XTv[:, 0], xT[:, 0])
            nc.sync.dma_start(WQKv[:], wqk[:])
            nc.sync.dma_start(XTv[:, 1], xT[:, 1])
            nc.sync.dma_start(XTv[:, 2], xT[:, 2])
            nc.sync.dma_start(XTv[:, 3], xT[:, 3])
            nc.sync.dma_start(WOv[:], wo[:])
            nc.scalar.dma_start(B0[:], b0v[:])
            # EM now; EB bands are issued lazily inside the blk0 iterations
            # (one per head-pair) so they never starve the prefix transfers
            nc.gpsimd.dma_start(EM[:], em0[:])

            # HAM warm-up: dummy matmuls ramp the PE p-state while DMAs land
            wps = aps.tile([128, 512], F32, name="wps", tag="projch", bufs=2)
            NWARM = 16
            for i in range(NWARM):
                nc.tensor.matmul(wps[:, 0:128], wrm[:, 0:128],
                                 wrm[:, 0:128],
                                 start=(i == 0), stop=(i == NWARM - 1))
            # DMA-paced warm-up: dummy matmuls gated on arriving wv chunks
            # keep a low-duty PE heartbeat through the DMA-fill window, so
            # the activity manager grants full clock before the projection
            # phase without the burst-then-claw-back pattern
            wps2 = aps.tile([128, 512], F32, name="wps2", tag="projch",
                            bufs=2)
            for k in range(8):
                nc.tensor.matmul(wps2[:], wvs[k][:, 0:128],
                                 wvs[k][:],
                                 start=(k == 0), stop=(k == 7))

            # ones columns for the denominator replicas (v halves get
            # overwritten by proj_b)
            for st in range(16):
                ones_v = VA[st].rearrange("p (h c) -> p h c", h=HPC)
                nc.gpsimd.memset(ones_v[:, :, 64:128], 1.0)

            # ---- projection b chunk: v[s, f] into VA v-halves -------------
            def proj_b_chunk(st, dve_copy=False):
                pv = aps.tile([128, 512], F32, name=f"pv{st}", tag="projch",
                              bufs=2)
                for kt in range(8):
                    nc.tensor.matmul(
                        pv[:],
                        xts(kt, 128 * st, 128),
                        wvs[kt][:],
                        start=(kt == 0), stop=(kt == 7))
                src = pv.rearrange("p (h c) -> p h c", h=HPC)
                dst = VA[st].rearrange("p (h c) -> p h c", h=HPC)
                if dve_copy:
                    nc.vector.tensor_copy(dst[:, :, 0:64], src[:])
                else:
                    nc.scalar.activation(dst[:, :, 0:64], src[:], AF.Copy)

            # ---- projection a: qkT[f, s], one (mt, sc) chunk at a time ----
            def proj_a_chunk(mt, sc):
                ps = aps.tile([128, 512], F32, name=f"pa{mt}_{sc}",
                              tag="projch", bufs=2)
                for kt in range(8):
                    nc.tensor.matmul(
                        ps[:],
                        wqks[kt][:, 128 * mt:128 * (mt + 1)],
                        xts(kt, 512 * sc, 512),
                        start=(kt == 0), stop=(kt == 7))
                # split the PSUM->SBUF casts across ACT and DVE to balance
                # engine load (GPSIMD cannot read PSUM)
                if (mt + sc) % 2 == 0:
                    nc.scalar.activation(qkT[mt][:, 512 * sc:512 * (sc + 1)],
                                         ps[:], AF.Copy)
                else:
                    nc.vector.tensor_copy(qkT[mt][:, 512 * sc:512 * (sc + 1)],
                                          ps[:])

            # ---- split-K out-projection: open partial sums over a subset of
            # head-pairs (their OT blocks are ready early), close with the
            # rest once the final head-pair lands ----
            po_hold = {}

            def outproj_open(st, kts, tag):
                blk_, qq = st // 4, st % 4
                if tag == "S":
                    po = aps.tile([128, 1024], F32, name=f"poH{st}", tag="S")
                    halves = [po[:, 0:512], po[:, 512:1024]]
                else:
                    halves = [aps.tile([128, 512], F32, name=f"poH{st}_{h}",
                                       tag="projch", bufs=2)[:]
                              for h in range(2)]
                po_hold[st] = halves
                for half in range(2):
                    for i, kt in enumerate(kts):
                        nc.tensor.matmul(
                            halves[half],
                            OT[4 * kt + blk_][:, 128 * qq:128 * (qq + 1)],
                            WO[kt][:, 512 * half:512 * half + 512],
                            start=(i == 0), stop=False,
                            skip_group_check=True)

            def outproj_close(st, kts):
                blk_, qq = st // 4, st % 4
                halves = po_hold[st]
                stg = pa.tile([128, 1024], F16, name=f"stgH{st}",
                              tag="stg", bufs=3)
                for half in range(2):
                    for i, kt in enumerate(kts):
                        nc.tensor.matmul(
                            halves[half],
                            OT[4 * kt + blk_][:, 128 * qq:128 * (qq + 1)],
                            WO[kt][:, 512 * half:512 * half + 512],
                            start=False, stop=(i == len(kts) - 1),
                            skip_group_check=True)
                    # tail: split the PSUM->SBUF casts across ACT and DVE
                    # so they drain in parallel after the last matmul
                    dst = stg[:, 512 * half:512 * half + 512]
                    if half == 0:
                        nc.scalar.activation(dst, halves[half], AF.Copy)
                    else:
                        nc.vector.tensor_copy(dst, halves[half])
                # one full-row 2KB-line DMA; strictly alternate queues in
                # emission order (14,12,13 -> sync,scalar,sync) so the tail
                # drains evenly on both
                eng = nc.scalar if st == 12 else nc.sync
                eng.dma_start(outp[128 * st:128 * (st + 1), :], stg[:])

            # ---- out-projection chunk (one s-tile) ----
            def outproj_chunk(st, split_stg=False):
                blk_, qq = st // 4, st % 4
                # both halves stage into one [128, 2KB] tile -> a single
                # full-row DMA (2KB lines = half the packets of 1KB lines)
                stg = pa.tile([128, 1024], F16, name=f"stg{st}",
                              tag="stg", bufs=3)
                for half in range(2):
                    po = aps.tile([128, 512], F32, name=f"po{st}_{half}",
                                  tag="projch", bufs=2)
                    for kt in range(4):
                        nc.tensor.matmul(
                            po[:],
                            OT[4 * kt + blk_][:, 128 * qq:128 * (qq + 1)],
                            WO[kt][:, 512 * half:512 * (half + 1)],
                            start=(kt == 0), stop=(kt == 3))
                    dst = stg[:, 512 * half:512 * (half + 1)]
                    if split_stg and half == 1:
                        nc.vector.tensor_copy(dst, po[:])
                    else:
                        nc.scalar.activation(dst, po[:], AF.Copy)
                eng = nc.scalar if (split_stg and st == 15) else nc.sync
                eng.dma_start(outp[128 * st:128 * (st + 1), :], stg[:])

            # serial prefix: only what (blk0, hp0) needs — proj_b st0-3 and
            # the two qk chunks for head-pair 0, upper halves first so the
            # first scores (jts 7/6) start as soon as possible.
            for st in range(4):
                proj_b_chunk(st)
            proj_a_chunk(0, 0)
            proj_a_chunk(4, 0)

            def attention_iter(hp, blk, fillers):
                # jt3 (full-width) leads the score order for blk>0: its exp
                # and band-mult then sit at the HEAD of the ACT/DVE queues,
                # so the first PV (which must be jt3 — full-column PSUM
                # init) is never blocked behind the previous iteration's
                # normalize chain on DVE
                jts = [3, 0, 1, 2, 4, 5, 6, 7] if blk > 0 else [4, 5, 6, 7]
                first_jt = 3 if blk > 0 else 4
                pv_order = [first_jt] + [j for j in jts if j != first_jt]
                # paired psum: cols [0:512) head 2hp, [512:1024) head 2hp+1
                # rows 0-63: o numerator, rows 64-127: denominator replicas
                Op = aps.tile([128, 1024], F32, name=f"O{hp}_{blk}",
                              tag="Opair", bufs=1)
                Pt = {}
                # 'p' (held split-K out-proj opens) must allocate their PSUM
                # ring slots after ALL of this iter's S allocations, so they
                # only emit after the score loop
                inline_iter = iter([f for f in fillers if f[0] != 'p'])
                deferred = [f for f in fillers if f[0] == 'p']

                def emit(f):
                    if f is None:
                        return
                    kind, arg = f
                    if kind == 'a':
                        proj_a_chunk(*arg)
                    elif kind == 'b':
                        proj_b_chunk(arg)
                    elif kind == 'bd':
                        proj_b_chunk(arg, dve_copy=True)
                    elif kind in ('p', 'q'):
                        outproj_open(*arg)
                    elif kind == 'os':
                        outproj_chunk(arg, split_stg=True)
                    else:
                        outproj_chunk(arg)

                def emit_next_filler(tail=False):
                    emit(next(inline_iter, None))
                    if tail:
                        for f in deferred:
                            emit(f)
                        deferred.clear()

                for gi in range(0, len(jts), 2):
                    for jt in jts[gi:gi + 2]:
                        q0, w = _qrange(jt)
                        gsb = (blk - 1) * 512 + 128 * jt
                        Sp = aps.tile([128, 1024], F32,
                                      name=f"S{hp}_{blk}_{jt}", tag="S")
                        for par in (0, 1):
                            nc.tensor.matmul(
                                Sp[:, 512 * par:512 * par + w],
                                qkT[4 + hp][64 * par:64 * par + 64,
                                            gsb:gsb + 128],
                                qkT[hp][64 * par:64 * par + 64,
                                        512 * blk + q0:512 * blk + q0 + w],
                                start=True, stop=True,
                                tile_position=(64 * par, 0),
                                skip_group_check=True)
                        P = Ppool.tile([128, 1024], BF16,
                                       name=f"P{hp}_{blk}_{jt}", tag="P")
                        c0 = q0 - 128 * jt + 896
                        Pv = P.rearrange("p (two c) -> p two c", two=2)
                        Sv = Sp.rearrange("p (two c) -> p two c", two=2)
                        if blk > 0:
                            # paired exp + bias-mul (one op for both heads);
                            # route some band-mults to the idle Pool engine
                            nc.scalar.activation(Pv[:, :, 0:w], Sv[:, :, 0:w],
                                                 AF.Exp)
                            EBv = EB[hp].rearrange("p (two c) -> p two c",
                                                   two=2)
                            # late-PV jts' band-mults go to the idle Pool
                            # engine to unload DVE (their PVs run last)
                            meng = nc.gpsimd if jt >= 5 else nc.vector
                            meng.tensor_tensor(
                                Pv[:, :, 0:w], Pv[:, :, 0:w],
                                EBv[:, :, c0:c0 + w], ALU.mult)
                        else:
                            for par in (0, 1):
                                idx = (2 * hp + par) * 4 + (jt - 4)
                                nc.scalar.activation(
                                    P[:, 512 * par:512 * par + w],
                                    Sp[:, 512 * par:512 * par + w], AF.Exp,
                                    bias=B0[:, idx:idx + 1])
                            EMv = EM.rearrange("p (two c) -> p two c", two=2)
                            meng = nc.gpsimd if jt >= 6 else nc.vector
                            meng.tensor_tensor(
                                Pv[:, :, 0:w], Pv[:, :, 0:w],
                                EMv[:, :, 0:w], ALU.mult)
                        Pt[jt] = (P, q0, w)
                    # PE filler while ACT/DVE chew on the exps/muls
                    emit_next_filler()
                if blk == 0:
                    # lazy EB band load: sits behind this iter's Pool mults
                    # in gpsimd program order, so it starts only after the
                    # prefix-critical input DMAs have drained; first use is
                    # (blk1, same hp), several iterations later
                    nc.gpsimd.dma_start(EBv4[:, hp, :], ebig[:, hp, :])
                for i in range(4):
                    emit_next_filler()
                for i, jt in enumerate(pv_order):
                    st = 4 * (blk - 1) + jt
                    for par in (0, 1):
                        P, q0, w = Pt[jt]
                        hl = 2 * hp + par
                        nc.tensor.matmul(
                            Op[:, 512 * par + q0:512 * par + q0 + w],
                            VA[st][:, 128 * hl:128 * hl + 128],
                            P[:, 512 * par:512 * par + w],
                            start=(i == 0), stop=(i == len(pv_order) - 1),
                            skip_group_check=True)
                # held split-K opens go here: their PSUM ring slots only
                # free after this iter's last exps, and the PE would other-
                # wise idle while the normalize chain below runs
                for f in deferred:
                    emit(f)
                deferred.clear()
                # normalize: rows 64:128 hold the denominator replicated;
                # bounce to SBUF (approx_fast can't read PSUM accumulator
                # bits), reciprocal, then one mult per half writes f16 OT
                dnf = pa.tile([64, 1024], F32, name=f"dn{hp}_{blk}",
                              tag="dnf", bufs=2)
                rcp = pa.tile([64, 1024], F32, name=f"rc{hp}_{blk}",
                              tag="rcp", bufs=2)
                ot = OT[4 * hp + blk]
                if blk == 3 and hp == 3:
                    # last iteration: normalize by q-column halves in
                    # close-need order — cols 256:512 (both heads) feed
                    # close14 (qq=2) and chunk15 (qq=3) first, so the tail
                    # closes start while cols 0:256 still normalize
                    Opv = Op.rearrange("p (two c) -> p two c", two=2)
                    dnv = dnf.rearrange("p (two c) -> p two c", two=2)
                    rcv = rcp.rearrange("p (two c) -> p two c", two=2)
                    for ci, c0 in enumerate((256, 0)):
                        src = Opv[64:128, :, c0:c0 + 256]
                        if ci == 0:
                            nc.scalar.activation(dnv[:, :, c0:c0 + 256],
                                                 src, AF.Copy)
                        else:
                            nc.vector.tensor_copy(dnv[:, :, c0:c0 + 256],
                                                  src)
                        nc.vector.reciprocal_approx_fast(
                            rcv[:, :, c0:c0 + 256], dnv[:, :, c0:c0 + 256])
                        for par in (0, 1):
                            nc.vector.tensor_tensor(
                                ot[64 * par:64 * par + 64, c0:c0 + 256],
                                Op[0:64,
                                   512 * par + c0:512 * par + c0 + 256],
                                rcp[0:64,
                                    512 * par + c0:512 * par + c0 + 256],
                                ALU.mult)
                else:
                    # on DVE, not ACT: an ACT-side copy here lands ahead of
                    # the NEXT iteration's exps and stalls its score pipeline
                    nc.vector.tensor_copy(dnf[:], Op[64:128, :])
                    nc.vector.reciprocal_approx_fast(rcp[:], dnf[:])
                    for par in (0, 1):
                        nc.vector.tensor_tensor(
                            ot[64 * par:64 * par + 64, :],
                            Op[0:64, 512 * par:512 * par + 512],
                            rcp[0:64, 512 * par:512 * par + 512], ALU.mult)

            # blk-outer order: each iteration's fillers produce exactly the
            # qk/v chunks the NEXT iteration's scores need (just-in-time),
            # plus out-proj of completed blocks
            filler_plan = {
                # blk0 carries no b-chunks: they'd stall on the late
                # xT[:,512:1024] DMA; (1,0) needs them only by its pv
                (0, 0): [('a', (1, 0)), ('a', (5, 0))],
                (0, 1): [('a', (2, 0)), ('a', (6, 0))],
                (0, 2): [('a', (3, 0)), ('a', (7, 0))],
                (0, 3): [('a', (0, 1)), ('a', (4, 1))],
                (1, 0): [('a', (1, 1)), ('a', (5, 1)), ('bd', 4), ('bd', 5),
                         ('bd', 6), ('bd', 7)],
                (1, 1): [('a', (2, 1)), ('a', (6, 1)), ('bd', 8), ('o', 0)],
                (1, 2): [('a', (3, 1)), ('a', (7, 1)), ('bd', 9), ('o', 1)],
                (1, 3): [('a', (0, 2)), ('a', (4, 2)), ('bd', 10), ('o', 2)],
                (2, 0): [('a', (1, 2)), ('a', (5, 2)), ('bd', 11), ('o', 3)],
                (2, 1): [('a', (2, 2)), ('a', (6, 2)), ('bd', 12), ('o', 4)],
                (2, 2): [('a', (3, 2)), ('a', (7, 2)), ('bd', 13), ('o', 5)],
                (2, 3): [('a', (0, 3)), ('a', (4, 3)), ('bd', 14), ('bd', 15)],
                (3, 0): [('a', (1, 3)), ('a', (5, 3)), ('o', 6), ('o', 7)],
                (3, 1): [('a', (2, 3)), ('a', (6, 3)), ('o', 8)],
                (3, 2): [('a', (3, 3)), ('a', (7, 3)), ('o', 9)],
                (3, 3): [('os', 10), ('os', 11),
                         ('q', (14, (0, 1, 2), 'projch')),
                         ('p', (12, (0, 1, 2), 'S')),
                         ('p', (13, (0, 1, 2), 'S'))],
            }
            for blk in range(4):
                for hp in range(4):
                    attention_iter(hp, blk, filler_plan[(blk, hp)])
            # close 14 first (its projch slots gate st15's accumulator),
            # then chunk15: both need only the upper-half normalize; the
            # 12/13 closes wait for the lower half and overlap 15's cast
            outproj_close(14, (3,))
            outproj_chunk(15, split_stg=True)
            outproj_close(12, (3,))
            outproj_close(13, (3,))

    nc.compile()
    return nc


_NC = None


def _get_nc():
    global _NC
    if _NC is None:
        _NC = _build_nc()
    return _NC


def _host_consts():
    slopes = np.exp2(-(np.arange(H, dtype=np.float64) + 1.0) * 8.0 / H)
    p = np.arange(128)[:, None]
    c = np.arange(1408)[None, :]
    delta = (c - p - 384).astype(np.float64)
    valid = (delta >= 0) & (delta <= 512)
    eb = np.zeros((H, 128, 1408), ml_dtypes.bfloat16)
    for h in range(H):
        vals = np.exp(slopes[h] * (delta - 512.0) - CM)
        eb[h] = np.where(valid, vals, 0.0).astype(ml_dtypes.bfloat16)
    cc = np.arange(512)[None, :]
    em0 = (cc >= p).astype(ml_dtypes.bfloat16)
    em0 = np.concatenate([em0, em0], axis=1)  # paired [128, 1024]
    # pair-interleaved bands: [g, hp, 128, 2*1408]
    ebp = np.zeros((2, 4, 128, 2816), ml_dtypes.bfloat16)
    for g in range(2):
        for hp in range(4):
            ebp[g, hp, :, 0:1408] = eb[8 * g + 2 * hp]
            ebp[g, hp, :, 1408:2816] = eb[8 * g + 2 * hp + 1]
    b0 = np.zeros((2, 128, 32), np.float32)  # per head-group
    for g in range(2):
        for hl in range(HPC):
            for jtl in range(4):
                b0[g, :, hl * 4 + jtl] = (
                    -slopes[8 * g + hl] * (128.0 * jtl + p[:, 0]) - CM)
    return slopes, ebp, em0, b0


def kernel(x, w_in, w_out):
    global LAST_RESULTS
    x = np.asarray(x, dtype=np.float32)
    w_in = np.asarray(w_in, dtype=np.float32)
    w_out = np.asarray(w_out, dtype=np.float32)

    nc = _get_nc()
    _, ebp, em0, b0 = _host_consts()

    def pack(a, nk):
        # [128*nk, C] -> [128, nk, C] (SBUF big-tile layout, one DMA each)
        return np.ascontiguousarray(
            a.reshape(nk, 128, a.shape[1]).transpose(1, 0, 2))

    in_maps = []
    for core in range(NCORES):
        b, g = divmod(core, 2)
        r0 = 512 * g
        w_qk = np.concatenate(
            [w_in[r0:r0 + 512] * 0.125,
             w_in[E + r0:E + r0 + 512]], axis=0).T.astype(np.float16)
        w_v = w_in[2 * E + r0:2 * E + r0 + 512].T.astype(np.float16)
        w_o = w_out[:, r0:r0 + 512].T.astype(np.float16)
        xTc = x[b].T.astype(np.float16)
        # [1024, 2048] -> [p, colblock, k, 512]
        xTp = np.ascontiguousarray(
            xTc.reshape(8, 128, 4, 512).transpose(1, 2, 0, 3))
        in_maps.append({
            "xT": xTp,
            "w_qk": pack(w_qk, 8),
            "w_v": pack(w_v, 8),
            "w_o": pack(w_o, 4),
            "expbig": np.ascontiguousarray(ebp[g].transpose(1, 0, 2)),
            "em0": em0,
            "b0v": np.ascontiguousarray(b0[g]),
        })

    res = run_bass_kernel_spmd(nc, in_maps, core_ids=list(range(NCORES)))
    LAST_RESULTS = res
    out = np.stack([
        res.results[2 * b]["out_p"].astype(np.float32)
        + res.results[2 * b + 1]["out_p"].astype(np.float32)
        for b in range(B)
    ])
    return out



# revision 74
# speedup vs baseline: 1.2098x; 1.0030x over previous
"""Trainium2 Bass kernel: sliding-window multihead attention w/ ALiBi.

Computation (per reference):
  qkv = x @ w_in.T ; q,k,v heads ; blocked sliding-window causal attention
  (window=512, ALiBi bias slope_h*(q_idx-kv_idx)) ; out = o @ w_out.T

Sharding: 8 cores = 4 batches x 2 head-groups (8 heads each). Each core
computes its batch's QKV for its heads, attention, and a partial out-proj
over its heads' columns. Host sums the two head-group partials per batch.

Softmax trick: P = exp(s_raw) * EXPBIG where EXPBIG = exp(bias - bound)
is a host-precomputed Toeplitz band (exact 0 outside the valid window).
The row-max subtraction is replaced by a static bound folded into EXPBIG
(block 0 uses a per-partition ACT bias instead). Each head's PV stationary
is [v (64 cols) | ones (64 cols)], so the PV matmul lands the softmax
denominator replicated across PSUM rows 64:128 and a single DVE divide
per half produces the normalized output tile.
"""

import os
import numpy as np
import ml_dtypes
from contextlib import ExitStack

import concourse.bass as bass
import concourse.bacc as bacc
import concourse.tile as tile
import concourse.mybir as mybir
from concourse.bass_utils import run_bass_kernel_spmd

F16 = mybir.dt.float16
BF16 = mybir.dt.bfloat16
F32 = mybir.dt.float32
AF = mybir.ActivationFunctionType
ALU = mybir.AluOpType

B, S, E = 4, 2048, 1024
H, D, WIN = 16, 64, 512
NB = S // WIN          # 4 blocks
HPC = 8                # heads per core
NCORES = 8
CM = 6.0               # softmax bound safety margin

LAST_RESULTS = None


def _qrange(jt):
    # valid q-column range for scores j-tile jt (window band)
    lo = max(0, 128 * jt - 512)
    hi = min(512, 128 * jt + 128)
    return lo, hi - lo


def _build_nc():
    nc = bacc.Bacc("TRN2", target_bir_lowering=False, debug=False,
                   num_devices=NCORES)

    # host pre-packs every tensor in its SBUF layout ([128, k, cols]) so
    # each one loads with a single large DMA: per-queue dma_start issue
    # overhead (~0.9us each) was the real input-phase bottleneck
    # xT packed [p, colblock, k, 512]: per-partition lines are 8KB contiguous
    # (k-major inside a column block), so each col-block loads as one fast DMA
    xT = nc.dram_tensor("xT", [128, 4, 8, 512], F16,
                        kind="ExternalInput").ap()
    wqk = nc.dram_tensor("w_qk", [128, 8, 1024], F16,
                         kind="ExternalInput").ap()
    wv = nc.dram_tensor("w_v", [128, 8, 512], F16, kind="ExternalInput").ap()
    wo = nc.dram_tensor("w_o", [128, 4, 1024], F16,
                        kind="ExternalInput").ap()
    ebig = nc.dram_tensor("expbig", [128, 4, 2816], BF16,
                          kind="ExternalInput").ap()
    em0 = nc.dram_tensor("em0", [128, 1024], BF16, kind="ExternalInput").ap()
    b0v = nc.dram_tensor("b0v", [128, 32], F32, kind="ExternalInput").ap()
    outp = nc.dram_tensor("out_p", [S, E], F16, kind="ExternalOutput").ap()

    with tile.TileContext(nc) as tc, ExitStack() as ctx:
        pp = ctx.enter_context(tc.tile_pool(name="persist", bufs=1))

        # persistent SBUF tensors
        qkT = [pp.tile([128, S], F16, name=f"qkT{m}", tag=f"qkT{m}")
               for m in range(8)]                       # f-major qk.T
        VA = [pp.tile([128, HPC * 128], BF16, name=f"VA{s}", tag=f"VA{s}")
              for s in range(16)]                  # per head: v(64) | ones(64)
        OT = [pp.tile([128, 512], F16, name=f"OT{i}", tag=f"OT{i}")
              for i in range(16)]                       # normalized o.T
        EBB = pp.tile([128, 4 * 2816], BF16, name="EBB", tag="EBB")
        EBv4 = EBB.rearrange("p (h c) -> p h c", h=4)
        EB = [EBv4[:, h, :] for h in range(4)]     # exp(bias-bound) band pairs
        EM = pp.tile([128, 1024], BF16, name="EM", tag="EM")  # blk0 causal 0/1
        B0 = pp.tile([128, 32], F32, name="B0", tag="B0")    # blk0 exp biases
        WOB = pp.tile([128, 4 * 1024], F16, name="WOB", tag="WOB")
        WOv = WOB.rearrange("p (k c) -> p k c", k=4)
        WO = [WOv[:, k, :] for k in range(4)]

        with tc.tile_pool(name="phA", bufs=1) as pa, \
             tc.tile_pool(name="Pp", bufs=8) as Ppool, \
             tc.tile_pool(name="aps", bufs=2, space="PSUM") as aps:
            XTB = pa.tile([128, 8 * S], F16, name="XTB", tag="XTB")
            XTv = XTB.rearrange("p (cb k r) -> p cb k r", cb=4, k=8)

            def xts(kt, c0, w):
                # xT column range [c0, c0+w) of chunk kt; must stay inside
                # one 512-col block
                cb, r = divmod(c0, 512)
                return XTv[:, cb, kt, r:r + w]
            WQKB = pa.tile([128, 8 * 1024], F16, name="WQKB", tag="WQKB")
            WQKv = WQKB.rearrange("p (k c) -> p k c", k=8)
            wqks = [WQKv[:, k, :] for k in range(8)]
            WVB = pa.tile([128, 8 * 512], F16, name="WVB", tag="WVB")
            WVv = WVB.rearrange("p (k c) -> p k c", k=8)
            wvs = [WVv[:, k, :] for k in range(8)]
            wrm = pa.tile([128, 256], F16, name="wrm", tag="wrm")

            # warm-up weights first so the PE can start ramping immediately
            nc.gpsimd.memset(wrm[:], 0.0)

            # -- input DMA: one large transfer per tensor / 512-col xT slice
            # (per-dma_start issue overhead ~0.9us was the input bottleneck;
            # host pre-packs so per-partition lines are >=8KB contiguous).
            # Single prioritized FIFO on the sync queue in strict need-order:
            # one queue alone sustains ~400GB/s, and parallel queues would
            # fair-share the bus and starve the prefix-critical transfers.
            nc.sync.dma_start(WVv[:], wv[:])
            nc.sync.dma_start(XTv[:, 0], xT[:, 0])
            nc.sync.dma_start(WQKv[:], wqk[:])
            nc.sync.dma_start(XTv[:, 1], xT[:, 1])
            nc.sync.dma_start(XTv[:, 2], xT[:, 2])
            nc.sync.dma_start(XTv[:, 3], xT[:, 3])
            nc.sync.dma_start(WOv[:], wo[:])
            nc.scalar.dma_start(B0[:], b0v[:])
            # EM now; EB bands are issued lazily inside the blk0 iterations
            # (one per head-pair) so they never starve the prefix transfers
            nc.gpsimd.dma_start(EM[:], em0[:])

            # HAM warm-up: dummy matmuls ramp the PE p-state while DMAs land
            wps = aps.tile([128, 512], F32, name="wps", tag="projch", bufs=2)
            NWARM = 16
            for i in range(NWARM):
                nc.tensor.matmul(wps[:, 0:128], wrm[:, 0:128],
                                 wrm[:, 0:128],
                                 start=(i == 0), stop=(i == NWARM - 1))
            # DMA-paced warm-up: dummy matmuls gated on arriving wv chunks
            # keep a low-duty PE heartbeat through the DMA-fill window, so
            # the activity manager grants full clock before the projection
            # phase without the burst-then-claw-back pattern
            wps2 = aps.tile([128, 512], F32, name="wps2", tag="projch",
                            bufs=2)
            for k in range(8):
                nc.tensor.matmul(wps2[:], wvs[k][:, 0:128],
                                 wvs[k][:],
                                 start=(k == 0), stop=(k == 7))

            # ones columns for the denominator replicas (v halves get
            # overwritten by proj_b)
            for st in range(16):
                ones_v = VA[st].rearrange("p (h c) -> p h c", h=HPC)
                nc.gpsimd.memset(ones_v[:, :, 64:128], 1.0)

            # ---- projection b chunk: v[s, f] into VA v-halves -------------
            def proj_b_chunk(st, dve_copy=False):
                pv = aps.tile([128, 512], F32, name=f"pv{st}", tag="projch",
                              bufs=2)
                for kt in range(8):
                    nc.tensor.matmul(
                        pv[:],
                        xts(kt, 128 * st, 128),
                        wvs[kt][:],
                        start=(kt == 0), stop=(kt == 7))
                src = pv.rearrange("p (h c) -> p h c", h=HPC)
                dst = VA[st].rearrange("p (h c) -> p h c", h=HPC)
                if dve_copy:
                    nc.vector.tensor_copy(dst[:, :, 0:64], src[:])
                else:
                    nc.scalar.activation(dst[:, :, 0:64], src[:], AF.Copy)

            # ---- projection a: qkT[f, s], one (mt, sc) chunk at a time ----
            def proj_a_chunk(mt, sc):
                ps = aps.tile([128, 512], F32, name=f"pa{mt}_{sc}",
                              tag="projch", bufs=2)
                for kt in range(8):
                    nc.tensor.matmul(
                        ps[:],
                        wqks[kt][:, 128 * mt:128 * (mt + 1)],
                        xts(kt, 512 * sc, 512),
                        start=(kt == 0), stop=(kt == 7))
                # split the PSUM->SBUF casts across ACT and DVE to balance
                # engine load (GPSIMD cannot read PSUM)
                if (mt + sc) % 2 == 0:
                    nc.scalar.activation(qkT[mt][:, 512 * sc:512 * (sc + 1)],
                                         ps[:], AF.Copy)
                else:
                    nc.vector.tensor_copy(qkT[mt][:, 512 * sc:512 * (sc + 1)],
                                          ps[:])

            # ---- split-K out-projection: open partial sums over a subset of
            # head-pairs (their OT blocks are ready early), close with the
            # rest once the final head-pair lands ----
            po_hold = {}

            def outproj_open(st, kts, tag):
                blk_, qq = st // 4, st % 4
                if tag == "S":
                    po = aps.tile([128, 1024], F32, name=f"poH{st}", tag="S")
                    halves = [po[:, 0:512], po[:, 512:1024]]
                else:
                    halves = [aps.tile([128, 512], F32, name=f"poH{st}_{h}",
                                       tag="projch", bufs=2)[:]
                              for h in range(2)]
                po_hold[st] = halves
                for half in range(2):
                    for i, kt in enumerate(kts):
                        nc.tensor.matmul(
                            halves[half],
                            OT[4 * kt + blk_][:, 128 * qq:128 * (qq + 1)],
                            WO[kt][:, 512 * half:512 * half + 512],
                            start=(i == 0), stop=False,
                            skip_group_check=True)

            def outproj_close(st, kts):
                blk_, qq = st // 4, st % 4
                halves = po_hold[st]
                stg = pa.tile([128, 1024], F16, name=f"stgH{st}",
                              tag="stg", bufs=3)
                for half in range(2):
                    for i, kt in enumerate(kts):
                        nc.tensor.matmul(
                            halves[half],
                            OT[4 * kt + blk_][:, 128 * qq:128 * (qq + 1)],
                            WO[kt][:, 512 * half:512 * half + 512],
                            start=False, stop=(i == len(kts) - 1),
                            skip_group_check=True)
                    # tail: split the PSUM->SBUF casts across ACT and DVE
                    # so they drain in parallel after the last matmul
                    dst = stg[:, 512 * half:512 * half + 512]
                    if half == 0:
                        nc.scalar.activation(dst, halves[half], AF.Copy)
                    else:
                        nc.vector.tensor_copy(dst, halves[half])
                # one full-row 2KB-line DMA; strictly alternate queues in
                # emission order (14,12,13 -> sync,scalar,sync) so the tail
                # drains evenly on both
                eng = nc.scalar if st == 12 else nc.sync
                eng.dma_start(outp[128 * st:128 * (st + 1), :], stg[:])

            # ---- out-projection chunk (one s-tile) ----
            def outproj_chunk(st, split_stg=False):
                blk_, qq = st // 4, st % 4
                # both halves stage into one [128, 2KB] tile -> a single
                # full-row DMA (2KB lines = half the packets of 1KB lines)
                stg = pa.tile([128, 1024], F16, name=f"stg{st}",
                              tag="stg", bufs=3)
                for half in range(2):
                    po = aps.tile([128, 512], F32, name=f"po{st}_{half}",
                                  tag="projch", bufs=2)
                    for kt in range(4):
                        nc.tensor.matmul(
                            po[:],
                            OT[4 * kt + blk_][:, 128 * qq:128 * (qq + 1)],
                            WO[kt][:, 512 * half:512 * (half + 1)],
                            start=(kt == 0), stop=(kt == 3))
                    dst = stg[:, 512 * half:512 * (half + 1)]
                    if split_stg and half == 1:
                        nc.vector.tensor_copy(dst, po[:])
                    else:
                        nc.scalar.activation(dst, po[:], AF.Copy)
                eng = nc.scalar if (split_stg and st == 15) else nc.sync
                eng.dma_start(outp[128 * st:128 * (st + 1), :], stg[:])

            # serial prefix: only what (blk0, hp0) needs — proj_b st0-3 and
            # the two qk chunks for head-pair 0, upper halves first so the
            # first scores (jts 7/6) start as soon as possible.
            for st in range(4):
                proj_b_chunk(st)
            proj_a_chunk(0, 0)
            proj_a_chunk(4, 0)

            def attention_iter(hp, blk, fillers):
                jts = list(range(8)) if blk > 0 else [4, 5, 6, 7]
                first_jt = 3 if blk > 0 else 4
                pv_order = [first_jt] + [j for j in jts if j != first_jt]
                # paired psum: cols [0:512) head 2hp, [512:1024) head 2hp+1
                # rows 0-63: o numerator, rows 64-127: denominator replicas
                Op = aps.tile([128, 1024], F32, name=f"O{hp}_{blk}",
                              tag="Opair", bufs=1)
                Pt = {}
                # 'p' (held split-K out-proj opens) must allocate their PSUM
                # ring slots after ALL of this iter's S allocations, so they
                # only emit after the score loop
                inline_iter = iter([f for f in fillers if f[0] != 'p'])
                deferred = [f for f in fillers if f[0] == 'p']

                def emit(f):
                    if f is None:
                        return
                    kind, arg = f
                    if kind == 'a':
                        proj_a_chunk(*arg)
                    elif kind == 'b':
                        proj_b_chunk(arg)
                    elif kind == 'bd':
                        proj_b_chunk(arg, dve_copy=True)
                    elif kind in ('p', 'q'):
                        outproj_open(*arg)
                    elif kind == 'os':
                        outproj_chunk(arg, split_stg=True)
                    else:
                        outproj_chunk(arg)

                def emit_next_filler(tail=False):
                    emit(next(inline_iter, None))
                    if tail:
                        for f in deferred:
                            emit(f)
                        deferred.clear()

                for gi in range(0, len(jts), 2):
                    for jt in jts[gi:gi + 2]:
                        q0, w = _qrange(jt)
                        gsb = (blk - 1) * 512 + 128 * jt
                        Sp = aps.tile([128, 1024], F32,
                                      name=f"S{hp}_{blk}_{jt}", tag="S")
                        for par in (0, 1):
                            nc.tensor.matmul(
                                Sp[:, 512 * par:512 * par + w],
                                qkT[4 + hp][64 * par:64 * par + 64,
                                            gsb:gsb + 128],
                                qkT[hp][64 * par:64 * par + 64,
                                        512 * blk + q0:512 * blk + q0 + w],
                                start=True, stop=True,
                                tile_position=(64 * par, 0),
                                skip_group_check=True)
                        P = Ppool.tile([128, 1024], BF16,
                                       name=f"P{hp}_{blk}_{jt}", tag="P")
                        c0 = q0 - 128 * jt + 896
                        Pv = P.rearrange("p (two c) -> p two c", two=2)
                        Sv = Sp.rearrange("p (two c) -> p two c", two=2)
                        if blk > 0:
                            # paired exp + bias-mul (one op for both heads);
                            # route some band-mults to the idle Pool engine
                            nc.scalar.activation(Pv[:, :, 0:w], Sv[:, :, 0:w],
                                                 AF.Exp)
                            EBv = EB[hp].rearrange("p (two c) -> p two c",
                                                   two=2)
                            # late-PV jts' band-mults go to the idle Pool
                            # engine to unload DVE (their PVs run last)
                            meng = nc.gpsimd if jt >= 5 else nc.vector
                            meng.tensor_tensor(
                                Pv[:, :, 0:w], Pv[:, :, 0:w],
                                EBv[:, :, c0:c0 + w], ALU.mult)
                        else:
                            for par in (0, 1):
                                idx = (2 * hp + par) * 4 + (jt - 4)
                                nc.scalar.activation(
                                    P[:, 512 * par:512 * par + w],
                                    Sp[:, 512 * par:512 * par + w], AF.Exp,
                                    bias=B0[:, idx:idx + 1])
                            EMv = EM.rearrange("p (two c) -> p two c", two=2)
                            meng = nc.gpsimd if jt >= 6 else nc.vector
                            meng.tensor_tensor(
                                Pv[:, :, 0:w], Pv[:, :, 0:w],
                                EMv[:, :, 0:w], ALU.mult)
                        Pt[jt] = (P, q0, w)
                    # PE filler while ACT/DVE chew on the exps/muls
                    emit_next_filler()
                if blk == 0:
                    # lazy EB band load: sits behind this iter's Pool mults
                    # in gpsimd program order, so it starts only after the
                    # prefix-critical input DMAs have drained; first use is
                    # (blk1, same hp), several iterations later
                    nc.gpsimd.dma_start(EBv4[:, hp, :], ebig[:, hp, :])
                for i in range(4):
                    emit_next_filler()
                for i, jt in enumerate(pv_order):
                    st = 4 * (blk - 1) + jt
                    for par in (0, 1):
                        P, q0, w = Pt[jt]
                        hl = 2 * hp + par
                        nc.tensor.matmul(
                            Op[:, 512 * par + q0:512 * par + q0 + w],
                            VA[st][:, 128 * hl:128 * hl + 128],
                            P[:, 512 * par:512 * par + w],
                            start=(i == 0), stop=(i == len(pv_order) - 1),
                            skip_group_check=True)
                # held split-K opens go here: their PSUM ring slots only
                # free after this iter's last exps, and the PE would other-
                # wise idle while the normalize chain below runs
                for f in deferred:
                    emit(f)
                deferred.clear()
                # normalize: rows 64:128 hold the denominator replicated;
                # bounce to SBUF (approx_fast can't read PSUM accumulator
                # bits), reciprocal, then one mult per half writes f16 OT
                dnf = pa.tile([64, 1024], F32, name=f"dn{hp}_{blk}",
                              tag="dnf", bufs=2)
                rcp = pa.tile([64, 1024], F32, name=f"rc{hp}_{blk}",
                              tag="rcp", bufs=2)
                ot = OT[4 * hp + blk]
                if blk == 3 and hp == 3:
                    # last iteration: normalize by q-column halves in
                    # close-need order — cols 256:512 (both heads) feed
                    # close14 (qq=2) and chunk15 (qq=3) first, so the tail
                    # closes start while cols 0:256 still normalize
                    Opv = Op.rearrange("p (two c) -> p two c", two=2)
                    dnv = dnf.rearrange("p (two c) -> p two c", two=2)
                    rcv = rcp.rearrange("p (two c) -> p two c", two=2)
                    for ci, c0 in enumerate((256, 0)):
                        src = Opv[64:128, :, c0:c0 + 256]
                        if ci == 0:
                            nc.scalar.activation(dnv[:, :, c0:c0 + 256],
                                                 src, AF.Copy)
                        else:
                            nc.vector.tensor_copy(dnv[:, :, c0:c0 + 256],
                                                  src)
                        nc.vector.reciprocal_approx_fast(
                            rcv[:, :, c0:c0 + 256], dnv[:, :, c0:c0 + 256])
                        for par in (0, 1):
                            nc.vector.tensor_tensor(
                                ot[64 * par:64 * par + 64, c0:c0 + 256],
                                Op[0:64,
                                   512 * par + c0:512 * par + c0 + 256],
                                rcp[0:64,
                                    512 * par + c0:512 * par + c0 + 256],
                                ALU.mult)
                else:
                    # on DVE, not ACT: an ACT-side copy here lands ahead of
                    # the NEXT iteration's exps and stalls its score pipeline
                    nc.vector.tensor_copy(dnf[:], Op[64:128, :])
                    nc.vector.reciprocal_approx_fast(rcp[:], dnf[:])
                    for par in (0, 1):
                        nc.vector.tensor_tensor(
                            ot[64 * par:64 * par + 64, :],
                            Op[0:64, 512 * par:512 * par + 512],
                            rcp[0:64, 512 * par:512 * par + 512], ALU.mult)

            # blk-outer order: each iteration's fillers produce exactly the
            # qk/v chunks the NEXT iteration's scores need (just-in-time),
            # plus out-proj of completed blocks
            filler_plan = {
                # blk0 carries no b-chunks: they'd stall on the late
                # xT[:,512:1024] DMA; (1,0) needs them only by its pv
                (0, 0): [('a', (1, 0)), ('a', (5, 0))],
                (0, 1): [('a', (2, 0)), ('a', (6, 0))],
                (0, 2): [('a', (3, 0)), ('a', (7, 0))],
                (0, 3): [('a', (0, 1)), ('a', (4, 1))],
                (1, 0): [('a', (1, 1)), ('a', (5, 1)), ('bd', 4), ('bd', 5),
                         ('bd', 6), ('bd', 7)],
                (1, 1): [('a', (2, 1)), ('a', (6, 1)), ('bd', 8), ('o', 0)],
                (1, 2): [('a', (3, 1)), ('a', (7, 1)), ('bd', 9), ('o', 1)],
                (1, 3): [('a', (0, 2)), ('a', (4, 2)), ('bd', 10), ('o', 2)],
                (2, 0): [('a', (1, 2)), ('a', (5, 2)), ('bd', 11), ('o', 3)],
                (2, 1): [('a', (2, 2)), ('a', (6, 2)), ('bd', 12), ('o', 4)],
                (2, 2): [('a', (3, 2)), ('a', (7, 2)), ('bd', 13), ('o', 5)],
                (2, 3): [('a', (0, 3)), ('a', (4, 3)), ('bd', 14), ('bd', 15)],
                (3, 0): [('a', (1, 3)), ('a', (5, 3)), ('o', 6), ('o', 7)],
                (3, 1): [('a', (2, 3)), ('a', (6, 3)), ('o', 8)],
                (3, 2): [('a', (3, 3)), ('a', (7, 3)), ('o', 9)],
                (3, 3): [('os', 10), ('os', 11),
                         ('q', (14, (0, 1, 2), 'projch')),
                         ('p', (12, (0, 1, 2), 'S')),
                         ('p', (13, (0, 1, 2), 'S'))],
            }
            for blk in range(4):
                for hp in range(4):
                    attention_iter(hp, blk, filler_plan[(blk, hp)])
            # close 14 first (its projch slots gate st15's accumulator),
            # then chunk15: both need only the upper-half normalize; the
            # 12/13 closes wait for the lower half and overlap 15's cast
            outproj_close(14, (3,))
            outproj_chunk(15, split_stg=True)
            outproj_close(12, (3,))
            outproj_close(13, (3,))

    nc.compile()
    return nc


_NC = None


def _get_nc():
    global _NC
    if _NC is None:
        _NC = _build_nc()
    return _NC


def _host_consts():
    slopes = np.exp2(-(np.arange(H, dtype=np.float64) + 1.0) * 8.0 / H)
    p = np.arange(128)[:, None]
    c = np.arange(1408)[None, :]
    delta = (c - p - 384).astype(np.float64)
    valid = (delta >= 0) & (delta <= 512)
    eb = np.zeros((H, 128, 1408), ml_dtypes.bfloat16)
    for h in range(H):
        vals = np.exp(slopes[h] * (delta - 512.0) - CM)
        eb[h] = np.where(valid, vals, 0.0).astype(ml_dtypes.bfloat16)
    cc = np.arange(512)[None, :]
    em0 = (cc >= p).astype(ml_dtypes.bfloat16)
    em0 = np.concatenate([em0, em0], axis=1)  # paired [128, 1024]
    # pair-interleaved bands: [g, hp, 128, 2*1408]
    ebp = np.zeros((2, 4, 128, 2816), ml_dtypes.bfloat16)
    for g in range(2):
        for hp in range(4):
            ebp[g, hp, :, 0:1408] = eb[8 * g + 2 * hp]
            ebp[g, hp, :, 1408:2816] = eb[8 * g + 2 * hp + 1]
    b0 = np.zeros((2, 128, 32), np.float32)  # per head-group
    for g in range(2):
        for hl in range(HPC):
            for jtl in range(4):
                b0[g, :, hl * 4 + jtl] = (
                    -slopes[8 * g + hl] * (128.0 * jtl + p[:, 0]) - CM)
    return slopes, ebp, em0, b0


def kernel(x, w_in, w_out):
    global LAST_RESULTS
    x = np.asarray(x, dtype=np.float32)
    w_in = np.asarray(w_in, dtype=np.float32)
    w_out = np.asarray(w_out, dtype=np.float32)

    nc = _get_nc()
    _, ebp, em0, b0 = _host_consts()

    def pack(a, nk):
        # [128*nk, C] -> [128, nk, C] (SBUF big-tile layout, one DMA each)
        return np.ascontiguousarray(
            a.reshape(nk, 128, a.shape[1]).transpose(1, 0, 2))

    in_maps = []
    for core in range(NCORES):
        b, g = divmod(core, 2)
        r0 = 512 * g
        w_qk = np.concatenate(
            [w_in[r0:r0 + 512] * 0.125,
             w_in[E + r0:E + r0 + 512]], axis=0).T.astype(np.float16)
        w_v = w_in[2 * E + r0:2 * E + r0 + 512].T.astype(np.float16)
        w_o = w_out[:, r0:r0 + 512].T.astype(np.float16)
        xTc = x[b].T.astype(np.float16)
        # [1024, 2048] -> [p, colblock, k, 512]
        xTp = np.ascontiguousarray(
            xTc.reshape(8, 128, 4, 512).transpose(1, 2, 0, 3))
        in_maps.append({
            "xT": xTp,
            "w_qk": pack(w_qk, 8),
            "w_v": pack(w_v, 8),
            "w_o": pack(w_o, 4),
            "expbig": np.ascontiguousarray(ebp[g].transpose(1, 0, 2)),
            "em0": em0,
            "b0v": np.ascontiguousarray(b0[g]),
        })

    res = run_bass_kernel_spmd(nc, in_maps, core_ids=list(range(NCORES)))
    LAST_RESULTS = res
    out = np.stack([
        res.results[2 * b]["out_p"].astype(np.float32)
        + res.results[2 * b + 1]["out_p"].astype(np.float32)
        for b in range(B)
    ])
    return out

